# revision 1
# baseline (speedup 1.0000x reference)
"""Trainium2 Bass kernel for nn_KnowledgeCriterion (ComplEx-style loss).

Full (unsharded) inputs:
  tri_feat_org: (256, 128, 1536) f32
  alpha:        (256, 64, 128)   f32
  mask:         (256, 64)        f32
Output: scalar f32 loss.

Strategy: data-parallel over batch on 8 NeuronCores (32 batches/core).
Each core computes three partial scalars (softplus-sum, regul-dot, mask-sum);
host combines:  loss = sp/numtrue + 0.01 * regul_dot/(B*S*R*D).

Per-batch on-chip pipeline (feature tile X = (R=128 part, F=1536 free)):
  s0[r] = sum_d [ r_re*(h_re*t_re + h_im*t_im) + r_im*(h_re*t_im - h_im*t_re) ]
     - DVE: A=h_re*t_re, B=h_im*t_im, then tensor_tensor_reduce chain with r_re
     - GPSIMD: Dp=h_re*t_im, Ep=h_im*t_re, scalar_tensor_tensor accums with r_im
  regul_dot += sum_r a2s[r] * sum_f X[r,f]^2
     - ACT Square -> X2; PE matmul (stationary=a2s col) accumulating in PSUM
  score = -(a^3)*s0, a=(alpha-0.1)*mask   (alpha transposed to (R,S) via PE)
  softplus(score) = (score+|score|)/2 + ln(1+exp(-|score|))
     - DVE tensor_scalar accum -> sum(score); GPSIMD STT max -> |score| + accum
     - ACT Exp(scale=-1), Ln(bias=1) + accum
"""
import numpy as np

B, S, R, F = 256, 64, 128, 1536
D = F // 6
N_CORES = 8
B_LOC = B // N_CORES

_CACHE = {}


def _build_nc(loop_k=1, ablate=()):
    """Build the per-core program. loop_k > 1 wraps the whole 32-batch body
    in a hardware For_i loop (timing-only variant: outputs stay correct
    because every rep restarts its accumulations). ablate: subset of
    {"squares", "s0", "softplus", "alpha", "dve_products"} — timing-only
    builds with that work removed (outputs then wrong)."""
    import contextlib
    import concourse.bacc as bacc
    import concourse.tile as tile
    import concourse.masks as masks
    from concourse import mybir

    F32 = mybir.dt.float32
    BF16 = mybir.dt.bfloat16
    ALU = mybir.AluOpType
    ACTF = mybir.ActivationFunctionType

    nc = bacc.Bacc("TRN2", target_bir_lowering=False, debug=False)
    feat = nc.dram_tensor("feat", [B_LOC, R, F], F32, kind="ExternalInput")
    alph = nc.dram_tensor("alpha", [B_LOC, S, R], F32, kind="ExternalInput")
    msk = nc.dram_tensor("mask", [B_LOC, S], F32, kind="ExternalInput")
    outp = nc.dram_tensor("partials", [1, 4], F32, kind="ExternalOutput")

    with tile.TileContext(nc) as tc:
        with (
            tc.tile_pool(name="const", bufs=1) as constp,
            tc.tile_pool(name="xf", bufs=6) as xf,
            tc.tile_pool(name="x2", bufs=3) as x2p,
            tc.tile_pool(name="prod", bufs=4) as prod,
            tc.tile_pool(name="alp", bufs=4) as alp,
            tc.tile_pool(name="sco", bufs=4) as sco,
            tc.tile_pool(name="cols", bufs=6) as colsp,
            tc.tile_pool(name="accum", bufs=1) as accp,
            tc.tile_pool(name="fin", bufs=1) as finp,
            tc.tile_pool(name="pst", bufs=3, space="PSUM") as pst,
            tc.tile_pool(name="psr", bufs=1, space="PSUM") as psr,
            tc.tile_pool(name="psf", bufs=1, space="PSUM") as psf,
            tc.tile_pool(name="psm", bufs=1, space="PSUM") as psm,
        ):
            ident = constp.tile([128, 128], F32)
            masks.make_identity(nc, ident[:])
            ones = constp.tile([128, 1], F32)
            nc.gpsimd.memset(ones[:], 1.0)

            # accumulation buffers: one column per batch
            lsums = accp.tile([128, B_LOC], F32)
            xsums = accp.tile([128, B_LOC], F32)
            absums = accp.tile([128, B_LOC], F32)
            if "softplus" in ablate:
                for t in (lsums, xsums, absums):
                    nc.gpsimd.memset(t[:], 0.0)

            # one consolidated mask load (B_LOC,S) -> transpose -> (S,B_LOC)
            mask_nat = accp.tile([B_LOC, S], F32)
            nc.sync.dma_start(mask_nat[:], msk.ap())
            maskT_ps = psm.tile([S, B_LOC], F32, tag="maskT_ps")
            nc.tensor.transpose(maskT_ps[:], mask_nat[:], ident[:B_LOC, :B_LOC])
            mask_cols = accp.tile([S, B_LOC], F32)
            nc.vector.tensor_copy(mask_cols[:], maskT_ps[:])
            m01 = accp.tile([S, B_LOC], F32)
            nc.vector.tensor_scalar(
                out=m01[:], in0=mask_cols[:], scalar1=-0.1, scalar2=0.0,
                op0=ALU.mult, op1=ALU.add)

            # persistent PSUM accumulators for regul (3 chunks of 512)
            if "squares" not in ablate:
                rg_ps = [psr.tile([1, 512], F32, name=f"rg_ps{k}", tag=f"rg{k}")
                         for k in range(3)]

            if loop_k > 1:
                loop_cm = tc.For_i(
                    0, loop_k, 1,
                    hint_engines=(mybir.EngineType.DVE, mybir.EngineType.Activation,
                                  mybir.EngineType.Pool, mybir.EngineType.PE,
                                  mybir.EngineType.SP))
            else:
                loop_cm = contextlib.nullcontext()
            with loop_cm:
                for b in range(B_LOC):
                    # ---- loads ----
                    X = xf.tile([R, F], F32)
                    nc.sync.dma_start(X[:], feat.ap()[b])
                    alt = alp.tile([S, R], F32)
                    nc.sync.dma_start(alt[:], alph.ap()[b])

                    h_re = X[:, 0 * D:1 * D]
                    h_im = X[:, 1 * D:2 * D]
                    r_re = X[:, 2 * D:3 * D]
                    r_im = X[:, 3 * D:4 * D]
                    t_re = X[:, 4 * D:5 * D]
                    t_im = X[:, 5 * D:6 * D]

                    # ---- alpha side ----
                    am = alp.tile([S, R], F32, tag="am")
                    nc.vector.tensor_scalar(
                        out=am[:], in0=alt[:], scalar1=0.1, scalar2=mask_cols[:, b:b + 1],
                        op0=ALU.subtract, op1=ALU.mult)
                    amT_ps = pst.tile([R, S], F32, tag="amT_ps")
                    nc.tensor.transpose(amT_ps[:], am[:], ident[:S, :S])
                    amT = alp.tile([R, S], F32, tag="amT")
                    nc.scalar.copy(amT[:], amT_ps[:])

                    a2T = sco.tile([R, S], F32, tag="a2T")
                    a2s = colsp.tile([R, 1], F32, tag="a2s")
                    nc.vector.scalar_tensor_tensor(
                        out=a2T[:], in0=amT[:], scalar=1.0, in1=amT[:],
                        op0=ALU.mult, op1=ALU.mult, accum_out=a2s[:])
                    a3T = sco.tile([R, S], F32, tag="a3T")
                    nc.vector.tensor_tensor(out=a3T[:], in0=a2T[:], in1=amT[:], op=ALU.mult)

                    # ---- feature side: squares for regul (bf16 for full-rate PE;
                    # regul is a 1e-4-scale term of the output so bf16 is ample) ----
                    if "squares" not in ablate:
                        a2sb = colsp.tile([R, 1], BF16, tag="a2sb")
                        nc.scalar.copy(a2sb[:], a2s[:])
                        X2 = x2p.tile([R, F], BF16)
                        nc.scalar.activation(out=X2[:], in_=X[:], func=ACTF.Square)
                        for k in range(3):
                            nc.tensor.matmul(
                                rg_ps[k][:], a2sb[:], X2[:, k * 512:(k + 1) * 512],
                                start=(b == 0), stop=(b == B_LOC - 1))

                    if "s0" in ablate:
                        negs0 = a2s
                    else:
                        # ---- feature side: s0 ----
                        # Pairing via slice adjacency in X:
                        #   AD = h_re * [t_re|t_im]        (one stride-0-bcast TT)
                        #   EB = h_im * [t_re|t_im] = [E|B]
                        #   p1 = sum(AD * [r_re|r_im])     (fused STT accum)
                        #   X[h_im slot] <- -r_im, then [negrim|r_re] is contiguous:
                        #   p2 = sum(EB * [-r_im|r_re]) = -E*r_im + B*r_re
                        #   negs0 = -(p1 + p2)
                        hre2 = X[:, None, 0:D].broadcast_to([R, 2, D])
                        him2 = X[:, None, D:2 * D].broadcast_to([R, 2, D])
                        tpair = X[:, 4 * D:6 * D]
                        AD = prod.tile([R, 2 * D], F32, tag="AD")
                        nc.gpsimd.tensor_tensor(out=AD[:], in0=hre2, in1=tpair,
                                                op=ALU.mult)
                        EB = prod.tile([R, 2 * D], F32, tag="EB")
                        nc.vector.tensor_tensor(out=EB[:], in0=him2, in1=tpair,
                                                op=ALU.mult)
                        nc.vector.tensor_scalar(
                            out=X[:, 1 * D:2 * D], in0=X[:, 3 * D:4 * D],
                            scalar1=-1.0, scalar2=0.0, op0=ALU.mult, op1=ALU.add)
                        jA = prod.tile([R, 2 * D], F32, tag="jA")
                        p1 = colsp.tile([R, 1], F32, tag="p1")
                        nc.vector.scalar_tensor_tensor(
                            out=jA[:], in0=AD[:], scalar=1.0, in1=X[:, 2 * D:4 * D],
                            op0=ALU.mult, op1=ALU.mult, accum_out=p1[:])
                        jB = prod.tile([R, 2 * D], F32, tag="jB")
                        p2 = colsp.tile([R, 1], F32, tag="p2")
                        nc.vector.scalar_tensor_tensor(
                            out=jB[:], in0=EB[:], scalar=1.0, in1=X[:, 1 * D:3 * D],
                            op0=ALU.mult, op1=ALU.mult, accum_out=p2[:])
                        negs0 = colsp.tile([R, 1], F32, tag="negs0")
                        nc.vector.scalar_tensor_tensor(
                            out=negs0[:], in0=p1[:], scalar=-1.0, in1=p2[:],
                            op0=ALU.mult, op1=ALU.subtract)
                    # ---- score & softplus ----
                        scoreT = sco.tile([R, S], F32, tag="scoreT")
                        nc.vector.tensor_scalar(
                            out=scoreT[:], in0=a3T[:], scalar1=negs0[:], scalar2=0.0,
                            op0=ALU.mult, op1=ALU.add, accum_out=xsums[:, b:b + 1])
                        mT = sco.tile([R, S], F32, tag="mT")
                        nc.scalar.activation(
                            out=mT[:], in_=scoreT[:], func=ACTF.Abs,
                            accum_out=absums[:, b:b + 1])
                        expT = sco.tile([R, S], F32, tag="expT")
                        nc.scalar.activation(out=expT[:], in_=mT[:], func=ACTF.Exp,
                                             scale=-1.0)
                        lnT = sco.tile([R, S], F32, tag="lnT")
                        nc.scalar.activation(
                            out=lnT[:], in_=expT[:], func=ACTF.Ln, bias=1.0,
                            accum_out=lsums[:, b:b + 1])

            # ---- endgame ----
            ALUm = ALU
            v = finp.tile([128, B_LOC], F32)
            nc.vector.tensor_tensor(out=v[:], in0=xsums[:], in1=absums[:],
                                    op=ALUm.add)
            v2 = finp.tile([128, B_LOC], F32)
            nc.vector.scalar_tensor_tensor(
                out=v2[:], in0=v[:], scalar=0.5, in1=lsums[:],
                op0=ALUm.mult, op1=ALUm.add)
            spv = finp.tile([128, 1], F32)
            nc.vector.tensor_reduce(
                out=spv[:], in_=v2[:], axis=mybir.AxisListType.X, op=ALUm.add)

            mred = finp.tile([S, 1], F32)
            nc.vector.tensor_reduce(
                out=mred[:], in_=mask_cols[:], axis=mybir.AxisListType.X, op=ALUm.add)

            rgs = finp.tile([1, 1], F32)
            if "squares" not in ablate:
                rgsb = finp.tile([1, F], F32)
                for k in range(3):
                    nc.scalar.copy(rgsb[:, k * 512:(k + 1) * 512], rg_ps[k][:])
                nc.vector.tensor_reduce(
                    out=rgs[:], in_=rgsb[:], axis=mybir.AxisListType.X, op=ALUm.add)
            else:
                nc.gpsimd.memset(rgs[:], 0.0)

            fin_ps = psf.tile([1, 4], F32)
            nc.tensor.matmul(fin_ps[:, 0:1], spv[:], ones[:], start=True, stop=True)
            nc.tensor.matmul(fin_ps[:, 2:3], mred[:], ones[:S, :], start=True, stop=True)

            out_sb = finp.tile([1, 4], F32)
            nc.scalar.copy(out_sb[:, 0:1], fin_ps[:, 0:1])
            nc.scalar.copy(out_sb[:, 1:2], rgs[:])
            nc.scalar.copy(out_sb[:, 2:3], fin_ps[:, 2:3])
            nc.gpsimd.memset(out_sb[:, 3:4], 0.0)
            nc.sync.dma_start(outp.ap(), out_sb[:])

    nc.compile()

    # Collapse the act-table loads: every activation used (square, abs, exp,
    # ln, copy, identity) lives in set 6 = natural_log_exp_and_others, but the
    # greedy inserter alternates sets 0/5 (one reload per iteration, ~1.3us
    # each). Pin the first load to set 6 and drop the rest (they carry no
    # sync info).
    first = True
    for bb in nc.m.functions[0].blocks:
        keep = []
        for inst in bb.instructions:
            if isinstance(inst, mybir.InstLoadActFuncSet):
                si = inst.sync_info
                assert not (si and (si.on_wait or si.on_update))
                if first:
                    inst.act_func_set_id = 6
                    first = False
                    keep.append(inst)
            else:
                keep.append(inst)
        if len(keep) != len(bb.instructions):
            il = bb.instructions
            il[:] = keep
    return nc


def _get_nc():
    if "nc" not in _CACHE:
        _CACHE["nc"] = _build_nc()
    return _CACHE["nc"]


def _get_runner():
    """Persistent jitted 8-core runner for the production build."""
    if "runner" in _CACHE:
        return _CACHE["runner"]
    _CACHE["runner"] = _make_runner(_get_nc())
    return _CACHE["runner"]


def _make_runner(nc):
    """Jitted 8-core runner (mirrors bass2jax.run_bass_via_pjrt)."""
    import jax
    from jax.sharding import Mesh, PartitionSpec
    from jax.experimental.shard_map import shard_map
    import concourse.mybir as mybir
    from concourse import bass2jax

    bass2jax.install_neuronx_cc_hook()

    partition_name = (nc.partition_id_tensor.name
                      if nc.partition_id_tensor else None)
    in_names, out_names, out_avals, zero_outs = [], [], [], []
    for alloc in nc.m.functions[0].allocations:
        if not isinstance(alloc, mybir.MemoryLocationSet):
            continue
        name = alloc.memorylocations[0].name
        if alloc.kind == "ExternalInput":
            if name != partition_name:
                in_names.append(name)
        elif alloc.kind == "ExternalOutput":
            out_names.append(name)
            shape = tuple(alloc.tensor_shape)
            dtype = mybir.dt.np(alloc.dtype)
            out_avals.append(jax.core.ShapedArray(shape, dtype))
            zero_outs.append(np.zeros(shape, dtype))
    n_params = len(in_names)
    all_names = in_names + out_names
    if partition_name is not None:
        all_names = all_names + [partition_name]

    def _body(*args):
        operands = list(args)
        if partition_name is not None:
            operands.append(bass2jax.partition_id_tensor())
        outs = bass2jax._bass_exec_p.bind(
            *operands,
            out_avals=tuple(out_avals),
            in_names=tuple(all_names),
            out_names=tuple(out_names),
            lowering_input_output_aliases=(),
            sim_require_finite=True,
            sim_require_nnan=True,
            nc=nc,
        )
        return tuple(outs)

    devices = jax.devices()[:N_CORES]
    mesh = Mesh(np.asarray(devices), ("core",))
    n_outs = len(out_names)
    sharded = jax.jit(
        shard_map(_body, mesh=mesh,
                  in_specs=(PartitionSpec("core"),) * (n_params + n_outs),
                  out_specs=(PartitionSpec("core"),) * n_outs,
                  check_rep=False),
        donate_argnums=tuple(range(n_params, n_params + n_outs)),
        keep_unused=True,
    )
    return {
        "fn": sharded, "mesh": mesh, "in_names": in_names,
        "out_names": out_names, "zero_outs": zero_outs, "n_params": n_params,
    }


def _shard_inputs(tri_feat_org, alpha, mask):
    """Concatenated per-core global inputs keyed by dram tensor name."""
    return {
        "feat": np.ascontiguousarray(tri_feat_org, dtype=np.float32),
        "alpha": np.ascontiguousarray(alpha, dtype=np.float32),
        "mask": np.ascontiguousarray(mask, dtype=np.float32),
    }


def _combine(partials_global):
    """partials_global: (8, 4) array of per-core partial scalars."""
    pg = np.asarray(partials_global, dtype=np.float64).reshape(N_CORES, 4)
    sp, rg, nt = pg[:, 0].sum(), pg[:, 1].sum(), pg[:, 2].sum()
    denom = float(B) * S * R * D
    return np.float32(sp / nt + 0.01 * rg / denom)


def kernel(tri_feat_org, alpha, mask):
    r = _get_runner()
    named = _shard_inputs(tri_feat_org, alpha, mask)
    args = [named[n] for n in r["in_names"]]
    zeros = [np.zeros((N_CORES * z.shape[0], *z.shape[1:]), z.dtype)
             for z in r["zero_outs"]]
    outs = r["fn"](*args, *zeros)
    part = np.asarray(outs[r["out_names"].index("partials")])
    return np.asarray(_combine(part), dtype=np.float32)



# revision 13
# speedup vs baseline: 67.9200x; 67.9200x over previous
"""Trainium2 Bass kernel for nn_KnowledgeCriterion (ComplEx-style loss).

Full (unsharded) inputs:
  tri_feat_org: (256, 128, 1536) f32
  alpha:        (256, 64, 128)   f32
  mask:         (256, 64)        f32
Output: scalar f32 loss.

Strategy: data-parallel over batch on 8 NeuronCores (32 batches/core).
Each core computes three partial scalars (softplus-sum, regul-dot, mask-sum);
host combines:  loss = sp/numtrue + 0.01 * regul_dot/(B*S*R*D).

Per-batch on-chip pipeline (feature tile X = (R=128 part, F=1536 free)):
  s0[r] = sum_d [ r_re*(h_re*t_re + h_im*t_im) + r_im*(h_re*t_im - h_im*t_re) ]
     - DVE: A=h_re*t_re, B=h_im*t_im, then tensor_tensor_reduce chain with r_re
     - GPSIMD: Dp=h_re*t_im, Ep=h_im*t_re, scalar_tensor_tensor accums with r_im
  regul_dot += sum_r a2s[r] * sum_f X[r,f]^2
     - ACT Square -> X2; PE matmul (stationary=a2s col) accumulating in PSUM
  score = -(a^3)*s0, a=(alpha-0.1)*mask   (alpha transposed to (R,S) via PE)
  softplus(score) = (score+|score|)/2 + ln(1+exp(-|score|))
     - DVE tensor_scalar accum -> sum(score); GPSIMD STT max -> |score| + accum
     - ACT Exp(scale=-1), Ln(bias=1) + accum
"""
import numpy as np

B, S, R, F = 256, 64, 128, 1536
D = F // 6
N_CORES = 8
B_LOC = B // N_CORES

_CACHE = {}


def _build_nc(loop_k=1, ablate=()):
    """Build the per-core program. loop_k > 1 wraps the whole 32-batch body
    in a hardware For_i loop (timing-only variant: outputs stay correct
    because every rep restarts its accumulations). ablate: subset of
    {"squares", "s0", "softplus", "alpha", "dve_products"} — timing-only
    builds with that work removed (outputs then wrong)."""
    import contextlib
    import concourse.bacc as bacc
    import concourse.tile as tile
    import concourse.masks as masks
    from concourse import mybir

    F32 = mybir.dt.float32
    BF16 = mybir.dt.bfloat16
    ALU = mybir.AluOpType
    ACTF = mybir.ActivationFunctionType

    nc = bacc.Bacc("TRN2", target_bir_lowering=False, debug=False)
    feat = nc.dram_tensor("feat", [B_LOC, R, F], F32, kind="ExternalInput")
    alph = nc.dram_tensor("alpha", [B_LOC, S, R], F32, kind="ExternalInput")
    msk = nc.dram_tensor("mask", [B_LOC, S], F32, kind="ExternalInput")
    outp = nc.dram_tensor("partials", [1, 4], F32, kind="ExternalOutput")

    with tile.TileContext(nc) as tc:
        with (
            tc.tile_pool(name="const", bufs=1) as constp,
            tc.tile_pool(name="xf", bufs=8) as xf,
            tc.tile_pool(name="x2", bufs=6) as x2p,
            tc.tile_pool(name="prod", bufs=8) as prod,
            tc.tile_pool(name="alp", bufs=8) as alp,
            tc.tile_pool(name="sco", bufs=12) as sco,
            tc.tile_pool(name="cols", bufs=12) as colsp,
            tc.tile_pool(name="accum", bufs=1) as accp,
            tc.tile_pool(name="fin", bufs=1) as finp,
            tc.tile_pool(name="pst", bufs=3, space="PSUM") as pst,
            tc.tile_pool(name="psr", bufs=1, space="PSUM") as psr,
            tc.tile_pool(name="psf", bufs=1, space="PSUM") as psf,
            tc.tile_pool(name="psm", bufs=1, space="PSUM") as psm,
        ):
            ident = constp.tile([128, 128], F32)
            masks.make_identity(nc, ident[:])
            ones = constp.tile([128, 1], F32)
            nc.gpsimd.memset(ones[:], 1.0)

            # accumulation buffers: one column per batch
            lsums = accp.tile([128, B_LOC], F32)
            xsums = accp.tile([128, B_LOC], F32)
            absums = accp.tile([128, B_LOC], F32)
            if "softplus" in ablate:
                for t in (lsums, xsums, absums):
                    nc.gpsimd.memset(t[:], 0.0)

            # one consolidated mask load (B_LOC,S) -> transpose -> (S,B_LOC)
            mask_nat = accp.tile([B_LOC, S], F32)
            nc.sync.dma_start(mask_nat[:], msk.ap())
            maskT_ps = psm.tile([S, B_LOC], F32, tag="maskT_ps")
            nc.tensor.transpose(maskT_ps[:], mask_nat[:], ident[:B_LOC, :B_LOC])
            mask_cols = accp.tile([S, B_LOC], F32)
            nc.vector.tensor_copy(mask_cols[:], maskT_ps[:])
            m01 = accp.tile([S, B_LOC], F32)
            nc.vector.tensor_scalar(
                out=m01[:], in0=mask_cols[:], scalar1=-0.1, scalar2=0.0,
                op0=ALU.mult, op1=ALU.add)

            # persistent PSUM accumulators for regul (3 chunks of 512)
            if "squares" not in ablate:
                rg_ps = [psr.tile([1, 512], F32, name=f"rg_ps{k}", tag=f"rg{k}")
                         for k in range(3)]

            if loop_k > 1:
                loop_cm = tc.For_i(
                    0, loop_k, 1,
                    hint_engines=(mybir.EngineType.DVE, mybir.EngineType.Activation,
                                  mybir.EngineType.Pool, mybir.EngineType.PE,
                                  mybir.EngineType.SP))
            else:
                loop_cm = contextlib.nullcontext()
            with loop_cm:
                for b in range(B_LOC):
                    # ---- loads ----
                    X = xf.tile([R, F], F32)
                    nc.sync.dma_start(X[:], feat.ap()[b])
                    alt = alp.tile([S, R], F32)
                    nc.sync.dma_start(alt[:], alph.ap()[b])

                    h_re = X[:, 0 * D:1 * D]
                    h_im = X[:, 1 * D:2 * D]
                    r_re = X[:, 2 * D:3 * D]
                    r_im = X[:, 3 * D:4 * D]
                    t_re = X[:, 4 * D:5 * D]
                    t_im = X[:, 5 * D:6 * D]

                    # ---- alpha side ----
                    am = alp.tile([S, R], F32, tag="am")
                    nc.vector.tensor_scalar(
                        out=am[:], in0=alt[:], scalar1=0.1, scalar2=mask_cols[:, b:b + 1],
                        op0=ALU.subtract, op1=ALU.mult)
                    amT_ps = pst.tile([R, S], F32, tag="amT_ps")
                    nc.tensor.transpose(amT_ps[:], am[:], ident[:S, :S])
                    amT = alp.tile([R, S], F32, tag="amT")
                    nc.scalar.copy(amT[:], amT_ps[:])

                    a2T = sco.tile([R, S], F32, tag="a2T")
                    a2s = colsp.tile([R, 1], F32, tag="a2s")
                    nc.vector.scalar_tensor_tensor(
                        out=a2T[:], in0=amT[:], scalar=1.0, in1=amT[:],
                        op0=ALU.mult, op1=ALU.mult, accum_out=a2s[:])
                    a3T = sco.tile([R, S], F32, tag="a3T")
                    nc.vector.tensor_tensor(out=a3T[:], in0=a2T[:], in1=amT[:], op=ALU.mult)

                    # ---- feature side: squares for regul (bf16 for full-rate PE;
                    # regul is a 1e-4-scale term of the output so bf16 is ample) ----
                    if "squares" not in ablate:
                        a2sb = colsp.tile([R, 1], BF16, tag="a2sb")
                        nc.scalar.copy(a2sb[:], a2s[:])
                        X2 = x2p.tile([R, F], BF16)
                        nc.scalar.activation(out=X2[:], in_=X[:], func=ACTF.Square)
                        for k in range(3):
                            nc.tensor.matmul(
                                rg_ps[k][:], a2sb[:], X2[:, k * 512:(k + 1) * 512],
                                start=(b == 0), stop=(b == B_LOC - 1))

                    if "s0" in ablate:
                        negs0 = a2s
                    else:
                        # ---- feature side: s0 ----
                        # Pairing via slice adjacency in X:
                        #   AD = h_re * [t_re|t_im]        (one stride-0-bcast TT)
                        #   EB = h_im * [t_re|t_im] = [E|B]
                        #   p1 = sum(AD * [r_re|r_im])     (fused STT accum)
                        #   X[h_im slot] <- -r_im, then [negrim|r_re] is contiguous:
                        #   p2 = sum(EB * [-r_im|r_re]) = -E*r_im + B*r_re
                        #   negs0 = -(p1 + p2)
                        hre2 = X[:, None, 0:D].broadcast_to([R, 2, D])
                        him2 = X[:, None, D:2 * D].broadcast_to([R, 2, D])
                        tpair = X[:, 4 * D:6 * D]
                        AD = prod.tile([R, 2 * D], F32, tag="AD")
                        nc.gpsimd.tensor_tensor(out=AD[:], in0=hre2, in1=tpair,
                                                op=ALU.mult)
                        EB = prod.tile([R, 2 * D], F32, tag="EB")
                        nc.vector.tensor_tensor(out=EB[:], in0=him2, in1=tpair,
                                                op=ALU.mult)
                        nc.vector.tensor_scalar(
                            out=X[:, 1 * D:2 * D], in0=X[:, 3 * D:4 * D],
                            scalar1=-1.0, scalar2=0.0, op0=ALU.mult, op1=ALU.add)
                        jA = prod.tile([R, 2 * D], F32, tag="jA")
                        p1 = colsp.tile([R, 1], F32, tag="p1")
                        nc.vector.scalar_tensor_tensor(
                            out=jA[:], in0=AD[:], scalar=1.0, in1=X[:, 2 * D:4 * D],
                            op0=ALU.mult, op1=ALU.mult, accum_out=p1[:])
                        jB = prod.tile([R, 2 * D], F32, tag="jB")
                        p2 = colsp.tile([R, 1], F32, tag="p2")
                        nc.vector.scalar_tensor_tensor(
                            out=jB[:], in0=EB[:], scalar=1.0, in1=X[:, 1 * D:3 * D],
                            op0=ALU.mult, op1=ALU.mult, accum_out=p2[:])
                        negs0 = colsp.tile([R, 1], F32, tag="negs0")
                        nc.vector.scalar_tensor_tensor(
                            out=negs0[:], in0=p1[:], scalar=-1.0, in1=p2[:],
                            op0=ALU.mult, op1=ALU.subtract)
                    # ---- score & softplus ----
                        scoreT = sco.tile([R, S], F32, tag="scoreT")
                        nc.vector.tensor_scalar(
                            out=scoreT[:], in0=a3T[:], scalar1=negs0[:], scalar2=0.0,
                            op0=ALU.mult, op1=ALU.add, accum_out=xsums[:, b:b + 1])
                        mT = sco.tile([R, S], F32, tag="mT")
                        nc.scalar.activation(
                            out=mT[:], in_=scoreT[:], func=ACTF.Abs,
                            accum_out=absums[:, b:b + 1])
                        expT = sco.tile([R, S], F32, tag="expT")
                        nc.scalar.activation(out=expT[:], in_=mT[:], func=ACTF.Exp,
                                             scale=-1.0)
                        lnT = sco.tile([R, S], F32, tag="lnT")
                        nc.scalar.activation(
                            out=lnT[:], in_=expT[:], func=ACTF.Ln, bias=1.0,
                            accum_out=lsums[:, b:b + 1])

            # ---- endgame ----
            ALUm = ALU
            v = finp.tile([128, B_LOC], F32)
            nc.vector.tensor_tensor(out=v[:], in0=xsums[:], in1=absums[:],
                                    op=ALUm.add)
            v2 = finp.tile([128, B_LOC], F32)
            nc.vector.scalar_tensor_tensor(
                out=v2[:], in0=v[:], scalar=0.5, in1=lsums[:],
                op0=ALUm.mult, op1=ALUm.add)
            spv = finp.tile([128, 1], F32)
            nc.vector.tensor_reduce(
                out=spv[:], in_=v2[:], axis=mybir.AxisListType.X, op=ALUm.add)

            mred = finp.tile([S, 1], F32)
            nc.vector.tensor_reduce(
                out=mred[:], in_=mask_cols[:], axis=mybir.AxisListType.X, op=ALUm.add)

            rgs = finp.tile([1, 1], F32)
            if "squares" not in ablate:
                rgsb = finp.tile([1, F], F32)
                for k in range(3):
                    nc.scalar.copy(rgsb[:, k * 512:(k + 1) * 512], rg_ps[k][:])
                nc.vector.tensor_reduce(
                    out=rgs[:], in_=rgsb[:], axis=mybir.AxisListType.X, op=ALUm.add)
            else:
                nc.gpsimd.memset(rgs[:], 0.0)

            fin_ps = psf.tile([1, 4], F32)
            nc.tensor.matmul(fin_ps[:, 0:1], spv[:], ones[:], start=True, stop=True)
            nc.tensor.matmul(fin_ps[:, 2:3], mred[:], ones[:S, :], start=True, stop=True)

            out_sb = finp.tile([1, 4], F32)
            nc.scalar.copy(out_sb[:, 0:1], fin_ps[:, 0:1])
            nc.scalar.copy(out_sb[:, 1:2], rgs[:])
            nc.scalar.copy(out_sb[:, 2:3], fin_ps[:, 2:3])
            nc.gpsimd.memset(out_sb[:, 3:4], 0.0)
            nc.sync.dma_start(outp.ap(), out_sb[:])

    nc.compile()

    # Collapse the act-table loads: every activation used (square, abs, exp,
    # ln, copy, identity) lives in set 6 = natural_log_exp_and_others, but the
    # greedy inserter alternates sets 0/5 (one reload per iteration, ~1.3us
    # each). Pin the first load to set 6 and drop the rest (they carry no
    # sync info).
    first = True
    for bb in nc.m.functions[0].blocks:
        keep = []
        for inst in bb.instructions:
            if isinstance(inst, mybir.InstLoadActFuncSet):
                si = inst.sync_info
                assert not (si and (si.on_wait or si.on_update))
                if first:
                    inst.act_func_set_id = 6
                    first = False
                    keep.append(inst)
            else:
                keep.append(inst)
        if len(keep) != len(bb.instructions):
            il = bb.instructions
            il[:] = keep
    return nc


def _get_nc():
    if "nc" not in _CACHE:
        _CACHE["nc"] = _build_nc()
    return _CACHE["nc"]


def _get_runner():
    """Persistent jitted 8-core runner for the production build."""
    if "runner" in _CACHE:
        return _CACHE["runner"]
    _CACHE["runner"] = _make_runner(_get_nc())
    return _CACHE["runner"]


def _make_runner(nc):
    """Jitted 8-core runner (mirrors bass2jax.run_bass_via_pjrt)."""
    import jax
    from jax.sharding import Mesh, PartitionSpec
    from jax.experimental.shard_map import shard_map
    import concourse.mybir as mybir
    from concourse import bass2jax

    bass2jax.install_neuronx_cc_hook()

    partition_name = (nc.partition_id_tensor.name
                      if nc.partition_id_tensor else None)
    in_names, out_names, out_avals, zero_outs = [], [], [], []
    for alloc in nc.m.functions[0].allocations:
        if not isinstance(alloc, mybir.MemoryLocationSet):
            continue
        name = alloc.memorylocations[0].name
        if alloc.kind == "ExternalInput":
            if name != partition_name:
                in_names.append(name)
        elif alloc.kind == "ExternalOutput":
            out_names.append(name)
            shape = tuple(alloc.tensor_shape)
            dtype = mybir.dt.np(alloc.dtype)
            out_avals.append(jax.core.ShapedArray(shape, dtype))
            zero_outs.append(np.zeros(shape, dtype))
    n_params = len(in_names)
    all_names = in_names + out_names
    if partition_name is not None:
        all_names = all_names + [partition_name]

    def _body(*args):
        operands = list(args)
        if partition_name is not None:
            operands.append(bass2jax.partition_id_tensor())
        outs = bass2jax._bass_exec_p.bind(
            *operands,
            out_avals=tuple(out_avals),
            in_names=tuple(all_names),
            out_names=tuple(out_names),
            lowering_input_output_aliases=(),
            sim_require_finite=True,
            sim_require_nnan=True,
            nc=nc,
        )
        return tuple(outs)

    devices = jax.devices()[:N_CORES]
    mesh = Mesh(np.asarray(devices), ("core",))
    n_outs = len(out_names)
    sharded = jax.jit(
        shard_map(_body, mesh=mesh,
                  in_specs=(PartitionSpec("core"),) * (n_params + n_outs),
                  out_specs=(PartitionSpec("core"),) * n_outs,
                  check_rep=False),
        donate_argnums=tuple(range(n_params, n_params + n_outs)),
        keep_unused=True,
    )
    return {
        "fn": sharded, "mesh": mesh, "in_names": in_names,
        "out_names": out_names, "zero_outs": zero_outs, "n_params": n_params,
    }


def _shard_inputs(tri_feat_org, alpha, mask):
    """Concatenated per-core global inputs keyed by dram tensor name."""
    return {
        "feat": np.ascontiguousarray(tri_feat_org, dtype=np.float32),
        "alpha": np.ascontiguousarray(alpha, dtype=np.float32),
        "mask": np.ascontiguousarray(mask, dtype=np.float32),
    }


def _combine(partials_global):
    """partials_global: (8, 4) array of per-core partial scalars."""
    pg = np.asarray(partials_global, dtype=np.float64).reshape(N_CORES, 4)
    sp, rg, nt = pg[:, 0].sum(), pg[:, 1].sum(), pg[:, 2].sum()
    denom = float(B) * S * R * D
    return np.float32(sp / nt + 0.01 * rg / denom)


def kernel(tri_feat_org, alpha, mask):
    r = _get_runner()
    named = _shard_inputs(tri_feat_org, alpha, mask)
    args = [named[n] for n in r["in_names"]]
    zeros = [np.zeros((N_CORES * z.shape[0], *z.shape[1:]), z.dtype)
             for z in r["zero_outs"]]
    outs = r["fn"](*args, *zeros)
    part = np.asarray(outs[r["out_names"].index("partials")])
    return np.asarray(_combine(part), dtype=np.float32)



# revision 16
# speedup vs baseline: 471.6658x; 6.9444x over previous
"""Trainium2 Bass kernel for nn_KnowledgeCriterion (ComplEx-style loss).

Full (unsharded) inputs:
  tri_feat_org: (256, 128, 1536) f32
  alpha:        (256, 64, 128)   f32
  mask:         (256, 64)        f32
Output: scalar f32 loss.

Strategy: data-parallel over batch on 8 NeuronCores (32 batches/core).
Each core computes three partial scalars (softplus-sum, regul-dot, mask-sum);
host combines:  loss = sp/numtrue + 0.01 * regul_dot/(B*S*R*D).

Per-batch on-chip pipeline (feature tile X = (R=128 part, F=1536 free)):
  s0[r] = sum_d [ r_re*(h_re*t_re + h_im*t_im) + r_im*(h_re*t_im - h_im*t_re) ]
     - DVE: A=h_re*t_re, B=h_im*t_im, then tensor_tensor_reduce chain with r_re
     - GPSIMD: Dp=h_re*t_im, Ep=h_im*t_re, scalar_tensor_tensor accums with r_im
  regul_dot += sum_r a2s[r] * sum_f X[r,f]^2
     - ACT Square -> X2; PE matmul (stationary=a2s col) accumulating in PSUM
  score = -(a^3)*s0, a=(alpha-0.1)*mask   (alpha transposed to (R,S) via PE)
  softplus(score) = (score+|score|)/2 + ln(1+exp(-|score|))
     - DVE tensor_scalar accum -> sum(score); GPSIMD STT max -> |score| + accum
     - ACT Exp(scale=-1), Ln(bias=1) + accum
"""
import numpy as np

B, S, R, F = 256, 64, 128, 1536
D = F // 6
N_CORES = 8
B_LOC = B // N_CORES

_CACHE = {}


def _build_nc(loop_k=1, ablate=()):
    """Build the per-core program. loop_k > 1 wraps the whole 32-batch body
    in a hardware For_i loop (timing-only variant: outputs stay correct
    because every rep restarts its accumulations). ablate: subset of
    {"squares", "s0", "softplus", "alpha", "dve_products"} — timing-only
    builds with that work removed (outputs then wrong)."""
    import contextlib
    import concourse.bacc as bacc
    import concourse.tile as tile
    import concourse.masks as masks
    from concourse import mybir

    F32 = mybir.dt.float32
    BF16 = mybir.dt.bfloat16
    ALU = mybir.AluOpType
    ACTF = mybir.ActivationFunctionType

    nc = bacc.Bacc("TRN2", target_bir_lowering=False, debug=False)
    feat = nc.dram_tensor("feat", [B_LOC, R, F], F32, kind="ExternalInput")
    alph = nc.dram_tensor("alpha", [B_LOC, S, R], F32, kind="ExternalInput")
    msk = nc.dram_tensor("mask", [B_LOC, S], F32, kind="ExternalInput")
    outp = nc.dram_tensor("partials", [1, 4], F32, kind="ExternalOutput")

    with tile.TileContext(nc) as tc:
        with (
            tc.tile_pool(name="const", bufs=1) as constp,
            tc.tile_pool(name="xf", bufs=8) as xf,
            tc.tile_pool(name="x2", bufs=6) as x2p,
            tc.tile_pool(name="prod", bufs=8) as prod,
            tc.tile_pool(name="alp", bufs=8) as alp,
            tc.tile_pool(name="sco", bufs=12) as sco,
            tc.tile_pool(name="cols", bufs=12) as colsp,
            tc.tile_pool(name="accum", bufs=1) as accp,
            tc.tile_pool(name="fin", bufs=1) as finp,
            tc.tile_pool(name="pst", bufs=3, space="PSUM") as pst,
            tc.tile_pool(name="psr", bufs=1, space="PSUM") as psr,
            tc.tile_pool(name="psf", bufs=1, space="PSUM") as psf,
            tc.tile_pool(name="psm", bufs=1, space="PSUM") as psm,
        ):
            ident = constp.tile([128, 128], F32)
            masks.make_identity(nc, ident[:])
            ones = constp.tile([128, 1], F32)
            nc.gpsimd.memset(ones[:], 1.0)

            # accumulation buffers: one column per batch
            lsums = accp.tile([128, B_LOC], F32)
            xsums = accp.tile([128, B_LOC], F32)
            absums = accp.tile([128, B_LOC], F32)
            if "softplus" in ablate:
                for t in (lsums, xsums, absums):
                    nc.gpsimd.memset(t[:], 0.0)

            # one consolidated mask load (B_LOC,S) -> transpose -> (S,B_LOC)
            mask_nat = accp.tile([B_LOC, S], F32)
            nc.sync.dma_start(mask_nat[:], msk.ap())
            maskT_ps = psm.tile([S, B_LOC], F32, tag="maskT_ps")
            nc.tensor.transpose(maskT_ps[:], mask_nat[:], ident[:B_LOC, :B_LOC])
            mask_cols = accp.tile([S, B_LOC], F32)
            nc.vector.tensor_copy(mask_cols[:], maskT_ps[:])
            m01 = accp.tile([S, B_LOC], F32)
            nc.vector.tensor_scalar(
                out=m01[:], in0=mask_cols[:], scalar1=-0.1, scalar2=0.0,
                op0=ALU.mult, op1=ALU.add)

            # persistent PSUM accumulators for regul (3 chunks of 512)
            if "squares" not in ablate:
                rg_ps = [psr.tile([1, 512], F32, name=f"rg_ps{k}", tag=f"rg{k}")
                         for k in range(3)]

            if loop_k > 1:
                loop_cm = tc.For_i(
                    0, loop_k, 1,
                    hint_engines=(mybir.EngineType.DVE, mybir.EngineType.Activation,
                                  mybir.EngineType.Pool, mybir.EngineType.PE,
                                  mybir.EngineType.SP))
            else:
                loop_cm = contextlib.nullcontext()
            with loop_cm:
                for b in range(B_LOC):
                    # ---- loads ----
                    X = xf.tile([R, F], F32)
                    nc.sync.dma_start(X[:], feat.ap()[b])
                    alt = alp.tile([S, R], F32)
                    nc.sync.dma_start(alt[:], alph.ap()[b])

                    h_re = X[:, 0 * D:1 * D]
                    h_im = X[:, 1 * D:2 * D]
                    r_re = X[:, 2 * D:3 * D]
                    r_im = X[:, 3 * D:4 * D]
                    t_re = X[:, 4 * D:5 * D]
                    t_im = X[:, 5 * D:6 * D]

                    # ---- alpha side ----
                    am = alp.tile([S, R], F32, tag="am")
                    nc.vector.tensor_scalar(
                        out=am[:], in0=alt[:], scalar1=0.1, scalar2=mask_cols[:, b:b + 1],
                        op0=ALU.subtract, op1=ALU.mult)
                    amT_ps = pst.tile([R, S], F32, tag="amT_ps")
                    nc.tensor.transpose(amT_ps[:], am[:], ident[:S, :S])
                    amT = alp.tile([R, S], F32, tag="amT")
                    nc.scalar.copy(amT[:], amT_ps[:])

                    a2T = sco.tile([R, S], F32, tag="a2T")
                    a2s = colsp.tile([R, 1], F32, tag="a2s")
                    nc.vector.scalar_tensor_tensor(
                        out=a2T[:], in0=amT[:], scalar=1.0, in1=amT[:],
                        op0=ALU.mult, op1=ALU.mult, accum_out=a2s[:])
                    a3T = sco.tile([R, S], F32, tag="a3T")
                    nc.vector.tensor_tensor(out=a3T[:], in0=a2T[:], in1=amT[:], op=ALU.mult)

                    # ---- feature side: squares for regul (bf16 for full-rate PE;
                    # regul is a 1e-4-scale term of the output so bf16 is ample) ----
                    if "squares" not in ablate:
                        a2sb = colsp.tile([R, 1], BF16, tag="a2sb")
                        nc.scalar.copy(a2sb[:], a2s[:])
                        X2 = x2p.tile([R, F], BF16)
                        nc.scalar.activation(out=X2[:], in_=X[:], func=ACTF.Square)
                        for k in range(3):
                            nc.tensor.matmul(
                                rg_ps[k][:], a2sb[:], X2[:, k * 512:(k + 1) * 512],
                                start=(b == 0), stop=(b == B_LOC - 1))

                    if "s0" in ablate:
                        negs0 = a2s
                    else:
                        # ---- feature side: s0 ----
                        # Pairing via slice adjacency in X:
                        #   AD = h_re * [t_re|t_im]        (one stride-0-bcast TT)
                        #   EB = h_im * [t_re|t_im] = [E|B]
                        #   p1 = sum(AD * [r_re|r_im])     (fused STT accum)
                        #   X[h_im slot] <- -r_im, then [negrim|r_re] is contiguous:
                        #   p2 = sum(EB * [-r_im|r_re]) = -E*r_im + B*r_re
                        #   negs0 = -(p1 + p2)
                        hre2 = X[:, None, 0:D].broadcast_to([R, 2, D])
                        him2 = X[:, None, D:2 * D].broadcast_to([R, 2, D])
                        tpair = X[:, 4 * D:6 * D]
                        AD = prod.tile([R, 2 * D], F32, tag="AD")
                        nc.gpsimd.tensor_tensor(out=AD[:], in0=hre2, in1=tpair,
                                                op=ALU.mult)
                        EB = prod.tile([R, 2 * D], F32, tag="EB")
                        nc.vector.tensor_tensor(out=EB[:], in0=him2, in1=tpair,
                                                op=ALU.mult)
                        nc.vector.tensor_scalar(
                            out=X[:, 1 * D:2 * D], in0=X[:, 3 * D:4 * D],
                            scalar1=-1.0, scalar2=0.0, op0=ALU.mult, op1=ALU.add)
                        jA = prod.tile([R, 2 * D], F32, tag="jA")
                        p1 = colsp.tile([R, 1], F32, tag="p1")
                        nc.vector.scalar_tensor_tensor(
                            out=jA[:], in0=AD[:], scalar=1.0, in1=X[:, 2 * D:4 * D],
                            op0=ALU.mult, op1=ALU.mult, accum_out=p1[:])
                        jB = prod.tile([R, 2 * D], F32, tag="jB")
                        p2 = colsp.tile([R, 1], F32, tag="p2")
                        nc.vector.scalar_tensor_tensor(
                            out=jB[:], in0=EB[:], scalar=1.0, in1=X[:, 1 * D:3 * D],
                            op0=ALU.mult, op1=ALU.mult, accum_out=p2[:])
                        negs0 = colsp.tile([R, 1], F32, tag="negs0")
                        nc.vector.scalar_tensor_tensor(
                            out=negs0[:], in0=p1[:], scalar=-1.0, in1=p2[:],
                            op0=ALU.mult, op1=ALU.subtract)
                    # ---- score & softplus ----
                        scoreT = sco.tile([R, S], F32, tag="scoreT")
                        nc.vector.tensor_scalar(
                            out=scoreT[:], in0=a3T[:], scalar1=negs0[:], scalar2=0.0,
                            op0=ALU.mult, op1=ALU.add, accum_out=xsums[:, b:b + 1])
                        mT = sco.tile([R, S], F32, tag="mT")
                        nc.scalar.activation(
                            out=mT[:], in_=scoreT[:], func=ACTF.Abs,
                            accum_out=absums[:, b:b + 1])
                        expT = sco.tile([R, S], F32, tag="expT")
                        nc.scalar.activation(out=expT[:], in_=mT[:], func=ACTF.Exp,
                                             scale=-1.0)
                        lnT = sco.tile([R, S], F32, tag="lnT")
                        nc.scalar.activation(
                            out=lnT[:], in_=expT[:], func=ACTF.Ln, bias=1.0,
                            accum_out=lsums[:, b:b + 1])

            # ---- endgame ----
            ALUm = ALU
            v = finp.tile([128, B_LOC], F32)
            nc.vector.tensor_tensor(out=v[:], in0=xsums[:], in1=absums[:],
                                    op=ALUm.add)
            v2 = finp.tile([128, B_LOC], F32)
            nc.vector.scalar_tensor_tensor(
                out=v2[:], in0=v[:], scalar=0.5, in1=lsums[:],
                op0=ALUm.mult, op1=ALUm.add)
            spv = finp.tile([128, 1], F32)
            nc.vector.tensor_reduce(
                out=spv[:], in_=v2[:], axis=mybir.AxisListType.X, op=ALUm.add)

            mred = finp.tile([S, 1], F32)
            nc.vector.tensor_reduce(
                out=mred[:], in_=mask_cols[:], axis=mybir.AxisListType.X, op=ALUm.add)

            rgs = finp.tile([1, 1], F32)
            if "squares" not in ablate:
                rgsb = finp.tile([1, F], F32)
                for k in range(3):
                    nc.scalar.copy(rgsb[:, k * 512:(k + 1) * 512], rg_ps[k][:])
                nc.vector.tensor_reduce(
                    out=rgs[:], in_=rgsb[:], axis=mybir.AxisListType.X, op=ALUm.add)
            else:
                nc.gpsimd.memset(rgs[:], 0.0)

            fin_ps = psf.tile([1, 4], F32)
            nc.tensor.matmul(fin_ps[:, 0:1], spv[:], ones[:], start=True, stop=True)
            nc.tensor.matmul(fin_ps[:, 2:3], mred[:], ones[:S, :], start=True, stop=True)

            out_sb = finp.tile([1, 4], F32)
            nc.scalar.copy(out_sb[:, 0:1], fin_ps[:, 0:1])
            nc.scalar.copy(out_sb[:, 1:2], rgs[:])
            nc.scalar.copy(out_sb[:, 2:3], fin_ps[:, 2:3])
            nc.gpsimd.memset(out_sb[:, 3:4], 0.0)
            nc.sync.dma_start(outp.ap(), out_sb[:])

    nc.compile()

    # Collapse the act-table loads: every activation used (square, abs, exp,
    # ln, copy, identity) lives in set 6 = natural_log_exp_and_others, but the
    # greedy inserter alternates sets 0/5 (one reload per iteration, ~1.3us
    # each). Pin the first load to set 6 and drop the rest (they carry no
    # sync info).
    first = True
    for bb in nc.m.functions[0].blocks:
        keep = []
        for inst in bb.instructions:
            if isinstance(inst, mybir.InstLoadActFuncSet):
                si = inst.sync_info
                assert not (si and (si.on_wait or si.on_update))
                if first:
                    inst.act_func_set_id = 6
                    first = False
                    keep.append(inst)
            else:
                keep.append(inst)
        if len(keep) != len(bb.instructions):
            il = bb.instructions
            il[:] = keep
    return nc


def _get_nc():
    if "nc" not in _CACHE:
        _CACHE["nc"] = _build_nc()
    return _CACHE["nc"]


def _get_runner():
    """Persistent jitted 8-core runner for the production build."""
    if "runner" in _CACHE:
        return _CACHE["runner"]
    _CACHE["runner"] = _make_runner(_get_nc())
    return _CACHE["runner"]


def _make_runner(nc):
    """Jitted 8-core runner (mirrors bass2jax.run_bass_via_pjrt)."""
    import jax
    from jax.sharding import Mesh, PartitionSpec
    from jax.experimental.shard_map import shard_map
    import concourse.mybir as mybir
    from concourse import bass2jax

    bass2jax.install_neuronx_cc_hook()

    partition_name = (nc.partition_id_tensor.name
                      if nc.partition_id_tensor else None)
    in_names, out_names, out_avals, zero_outs = [], [], [], []
    for alloc in nc.m.functions[0].allocations:
        if not isinstance(alloc, mybir.MemoryLocationSet):
            continue
        name = alloc.memorylocations[0].name
        if alloc.kind == "ExternalInput":
            if name != partition_name:
                in_names.append(name)
        elif alloc.kind == "ExternalOutput":
            out_names.append(name)
            shape = tuple(alloc.tensor_shape)
            dtype = mybir.dt.np(alloc.dtype)
            out_avals.append(jax.core.ShapedArray(shape, dtype))
            zero_outs.append(np.zeros(shape, dtype))
    n_params = len(in_names)
    all_names = in_names + out_names
    if partition_name is not None:
        all_names = all_names + [partition_name]

    def _body(*args):
        operands = list(args)
        if partition_name is not None:
            operands.append(bass2jax.partition_id_tensor())
        outs = bass2jax._bass_exec_p.bind(
            *operands,
            out_avals=tuple(out_avals),
            in_names=tuple(all_names),
            out_names=tuple(out_names),
            lowering_input_output_aliases=(),
            sim_require_finite=True,
            sim_require_nnan=True,
            nc=nc,
        )
        return tuple(outs)

    devices = jax.devices()[:N_CORES]
    mesh = Mesh(np.asarray(devices), ("core",))
    n_outs = len(out_names)
    sharded = jax.jit(
        shard_map(_body, mesh=mesh,
                  in_specs=(PartitionSpec("core"),) * (n_params + n_outs),
                  out_specs=(PartitionSpec("core"),) * n_outs,
                  check_rep=False),
        donate_argnums=tuple(range(n_params, n_params + n_outs)),
        keep_unused=True,
    )
    return {
        "fn": sharded, "mesh": mesh, "in_names": in_names,
        "out_names": out_names, "zero_outs": zero_outs, "n_params": n_params,
    }


def _shard_inputs(tri_feat_org, alpha, mask):
    """Concatenated per-core global inputs keyed by dram tensor name."""
    return {
        "feat": np.ascontiguousarray(tri_feat_org, dtype=np.float32),
        "alpha": np.ascontiguousarray(alpha, dtype=np.float32),
        "mask": np.ascontiguousarray(mask, dtype=np.float32),
    }


def _combine(partials_global):
    """partials_global: (8, 4) array of per-core partial scalars."""
    pg = np.asarray(partials_global, dtype=np.float64).reshape(N_CORES, 4)
    sp, rg, nt = pg[:, 0].sum(), pg[:, 1].sum(), pg[:, 2].sum()
    denom = float(B) * S * R * D
    return np.float32(sp / nt + 0.01 * rg / denom)


def kernel(tri_feat_org, alpha, mask):
    r = _get_runner()
    named = _shard_inputs(tri_feat_org, alpha, mask)
    args = [named[n] for n in r["in_names"]]
    zeros = [np.zeros((N_CORES * z.shape[0], *z.shape[1:]), z.dtype)
             for z in r["zero_outs"]]
    outs = r["fn"](*args, *zeros)
    part = np.asarray(outs[r["out_names"].index("partials")])
    return np.asarray(_combine(part), dtype=np.float32)



# revision 17
# speedup vs baseline: 476.6238x; 1.0105x over previous
"""Trainium2 Bass kernel for nn_KnowledgeCriterion (ComplEx-style loss).

Full (unsharded) inputs:
  tri_feat_org: (256, 128, 1536) f32
  alpha:        (256, 64, 128)   f32
  mask:         (256, 64)        f32
Output: scalar f32 loss.

Strategy: data-parallel over batch on 8 NeuronCores (32 batches/core).
Each core computes three partial scalars (softplus-sum, regul-dot, mask-sum);
host combines:  loss = sp/numtrue + 0.01 * regul_dot/(B*S*R*D).

Per-batch on-chip pipeline (feature tile X = (R=128 part, F=1536 free)):
  s0[r] = sum_d [ r_re*(h_re*t_re + h_im*t_im) + r_im*(h_re*t_im - h_im*t_re) ]
     - DVE: A=h_re*t_re, B=h_im*t_im, then tensor_tensor_reduce chain with r_re
     - GPSIMD: Dp=h_re*t_im, Ep=h_im*t_re, scalar_tensor_tensor accums with r_im
  regul_dot += sum_r a2s[r] * sum_f X[r,f]^2
     - ACT Square -> X2; PE matmul (stationary=a2s col) accumulating in PSUM
  score = -(a^3)*s0, a=(alpha-0.1)*mask   (alpha transposed to (R,S) via PE)
  softplus(score) = (score+|score|)/2 + ln(1+exp(-|score|))
     - DVE tensor_scalar accum -> sum(score); GPSIMD STT max -> |score| + accum
     - ACT Exp(scale=-1), Ln(bias=1) + accum
"""
import numpy as np

B, S, R, F = 256, 64, 128, 1536
D = F // 6
N_CORES = 8
B_LOC = B // N_CORES

_CACHE = {}


def _build_nc(loop_k=1, ablate=()):
    """Build the per-core program. loop_k > 1 wraps the whole 32-batch body
    in a hardware For_i loop (timing-only variant: outputs stay correct
    because every rep restarts its accumulations). ablate: subset of
    {"squares", "s0", "softplus", "alpha", "dve_products"} — timing-only
    builds with that work removed (outputs then wrong)."""
    import contextlib
    import concourse.bacc as bacc
    import concourse.tile as tile
    import concourse.masks as masks
    from concourse import mybir

    F32 = mybir.dt.float32
    BF16 = mybir.dt.bfloat16
    ALU = mybir.AluOpType
    ACTF = mybir.ActivationFunctionType

    nc = bacc.Bacc("TRN2", target_bir_lowering=False, debug=False)
    feat = nc.dram_tensor("feat", [B_LOC, R, F], F32, kind="ExternalInput")
    alph = nc.dram_tensor("alpha", [B_LOC, S, R], F32, kind="ExternalInput")
    msk = nc.dram_tensor("mask", [B_LOC, S], F32, kind="ExternalInput")
    outp = nc.dram_tensor("partials", [1, 4], F32, kind="ExternalOutput")

    with tile.TileContext(nc) as tc:
        with (
            tc.tile_pool(name="const", bufs=1) as constp,
            tc.tile_pool(name="xf", bufs=8) as xf,
            tc.tile_pool(name="x2", bufs=6) as x2p,
            tc.tile_pool(name="prod", bufs=8) as prod,
            tc.tile_pool(name="alp", bufs=12) as alp,
            tc.tile_pool(name="sco", bufs=16) as sco,
            tc.tile_pool(name="cols", bufs=16) as colsp,
            tc.tile_pool(name="accum", bufs=1) as accp,
            tc.tile_pool(name="fin", bufs=1) as finp,
            tc.tile_pool(name="pst", bufs=3, space="PSUM") as pst,
            tc.tile_pool(name="psr", bufs=1, space="PSUM") as psr,
            tc.tile_pool(name="psf", bufs=1, space="PSUM") as psf,
            tc.tile_pool(name="psm", bufs=1, space="PSUM") as psm,
        ):
            ident = constp.tile([128, 128], F32)
            masks.make_identity(nc, ident[:])
            ones = constp.tile([128, 1], F32)
            nc.gpsimd.memset(ones[:], 1.0)

            # accumulation buffers: one column per batch
            lsums = accp.tile([128, B_LOC], F32)
            xsums = accp.tile([128, B_LOC], F32)
            absums = accp.tile([128, B_LOC], F32)
            if "softplus" in ablate:
                for t in (lsums, xsums, absums):
                    nc.gpsimd.memset(t[:], 0.0)

            # one consolidated mask load (B_LOC,S) -> transpose -> (S,B_LOC)
            mask_nat = accp.tile([B_LOC, S], F32)
            nc.sync.dma_start(mask_nat[:], msk.ap())
            maskT_ps = psm.tile([S, B_LOC], F32, tag="maskT_ps")
            nc.tensor.transpose(maskT_ps[:], mask_nat[:], ident[:B_LOC, :B_LOC])
            mask_cols = accp.tile([S, B_LOC], F32)
            nc.vector.tensor_copy(mask_cols[:], maskT_ps[:])
            m01 = accp.tile([S, B_LOC], F32)
            nc.vector.tensor_scalar(
                out=m01[:], in0=mask_cols[:], scalar1=-0.1, scalar2=0.0,
                op0=ALU.mult, op1=ALU.add)

            # persistent PSUM accumulators for regul (3 chunks of 512)
            if "squares" not in ablate:
                rg_ps = [psr.tile([1, 512], F32, name=f"rg_ps{k}", tag=f"rg{k}")
                         for k in range(3)]

            if loop_k > 1:
                loop_cm = tc.For_i(
                    0, loop_k, 1,
                    hint_engines=(mybir.EngineType.DVE, mybir.EngineType.Activation,
                                  mybir.EngineType.Pool, mybir.EngineType.PE,
                                  mybir.EngineType.SP))
            else:
                loop_cm = contextlib.nullcontext()
            with loop_cm:
                for b in range(B_LOC):
                    # ---- loads ----
                    X = xf.tile([R, F], F32)
                    nc.sync.dma_start(X[:], feat.ap()[b])
                    alt = alp.tile([S, R], F32)
                    nc.sync.dma_start(alt[:], alph.ap()[b])

                    h_re = X[:, 0 * D:1 * D]
                    h_im = X[:, 1 * D:2 * D]
                    r_re = X[:, 2 * D:3 * D]
                    r_im = X[:, 3 * D:4 * D]
                    t_re = X[:, 4 * D:5 * D]
                    t_im = X[:, 5 * D:6 * D]

                    # ---- alpha side ----
                    am = alp.tile([S, R], F32, tag="am")
                    nc.vector.tensor_scalar(
                        out=am[:], in0=alt[:], scalar1=0.1, scalar2=mask_cols[:, b:b + 1],
                        op0=ALU.subtract, op1=ALU.mult)
                    amT_ps = pst.tile([R, S], F32, tag="amT_ps")
                    nc.tensor.transpose(amT_ps[:], am[:], ident[:S, :S])
                    amT = alp.tile([R, S], F32, tag="amT")
                    nc.scalar.copy(amT[:], amT_ps[:])

                    a2T = sco.tile([R, S], F32, tag="a2T")
                    a2s = colsp.tile([R, 1], F32, tag="a2s")
                    nc.vector.scalar_tensor_tensor(
                        out=a2T[:], in0=amT[:], scalar=1.0, in1=amT[:],
                        op0=ALU.mult, op1=ALU.mult, accum_out=a2s[:])
                    a3T = sco.tile([R, S], F32, tag="a3T")
                    nc.vector.tensor_tensor(out=a3T[:], in0=a2T[:], in1=amT[:], op=ALU.mult)

                    # ---- feature side: squares for regul (bf16 for full-rate PE;
                    # regul is a 1e-4-scale term of the output so bf16 is ample) ----
                    if "squares" not in ablate:
                        a2sb = colsp.tile([R, 1], BF16, tag="a2sb")
                        nc.scalar.copy(a2sb[:], a2s[:])
                        X2 = x2p.tile([R, F], BF16)
                        nc.scalar.activation(out=X2[:], in_=X[:], func=ACTF.Square)
                        for k in range(3):
                            nc.tensor.matmul(
                                rg_ps[k][:], a2sb[:], X2[:, k * 512:(k + 1) * 512],
                                start=(b == 0), stop=(b == B_LOC - 1))

                    if "s0" in ablate:
                        negs0 = a2s
                    else:
                        # ---- feature side: s0 ----
                        # Pairing via slice adjacency in X:
                        #   AD = h_re * [t_re|t_im]        (one stride-0-bcast TT)
                        #   EB = h_im * [t_re|t_im] = [E|B]
                        #   p1 = sum(AD * [r_re|r_im])     (fused STT accum)
                        #   X[h_im slot] <- -r_im, then [negrim|r_re] is contiguous:
                        #   p2 = sum(EB * [-r_im|r_re]) = -E*r_im + B*r_re
                        #   negs0 = -(p1 + p2)
                        hre2 = X[:, None, 0:D].broadcast_to([R, 2, D])
                        him2 = X[:, None, D:2 * D].broadcast_to([R, 2, D])
                        tpair = X[:, 4 * D:6 * D]
                        AD = prod.tile([R, 2 * D], F32, tag="AD")
                        nc.gpsimd.tensor_tensor(out=AD[:], in0=hre2, in1=tpair,
                                                op=ALU.mult)
                        EB = prod.tile([R, 2 * D], F32, tag="EB")
                        nc.vector.tensor_tensor(out=EB[:], in0=him2, in1=tpair,
                                                op=ALU.mult)
                        nc.vector.tensor_scalar(
                            out=X[:, 1 * D:2 * D], in0=X[:, 3 * D:4 * D],
                            scalar1=-1.0, scalar2=0.0, op0=ALU.mult, op1=ALU.add)
                        jA = prod.tile([R, 2 * D], F32, tag="jA")
                        p1 = colsp.tile([R, 1], F32, tag="p1")
                        nc.vector.scalar_tensor_tensor(
                            out=jA[:], in0=AD[:], scalar=1.0, in1=X[:, 2 * D:4 * D],
                            op0=ALU.mult, op1=ALU.mult, accum_out=p1[:])
                        jB = prod.tile([R, 2 * D], F32, tag="jB")
                        p2 = colsp.tile([R, 1], F32, tag="p2")
                        nc.vector.scalar_tensor_tensor(
                            out=jB[:], in0=EB[:], scalar=1.0, in1=X[:, 1 * D:3 * D],
                            op0=ALU.mult, op1=ALU.mult, accum_out=p2[:])
                        negs0 = colsp.tile([R, 1], F32, tag="negs0")
                        nc.vector.scalar_tensor_tensor(
                            out=negs0[:], in0=p1[:], scalar=-1.0, in1=p2[:],
                            op0=ALU.mult, op1=ALU.subtract)
                    # ---- score & softplus ----
                        scoreT = sco.tile([R, S], F32, tag="scoreT")
                        nc.vector.tensor_scalar(
                            out=scoreT[:], in0=a3T[:], scalar1=negs0[:], scalar2=0.0,
                            op0=ALU.mult, op1=ALU.add, accum_out=xsums[:, b:b + 1])
                        mT = sco.tile([R, S], F32, tag="mT")
                        nc.scalar.activation(
                            out=mT[:], in_=scoreT[:], func=ACTF.Abs,
                            accum_out=absums[:, b:b + 1])
                        expT = sco.tile([R, S], F32, tag="expT")
                        nc.scalar.activation(out=expT[:], in_=mT[:], func=ACTF.Exp,
                                             scale=-1.0)
                        lnT = sco.tile([R, S], F32, tag="lnT")
                        nc.scalar.activation(
                            out=lnT[:], in_=expT[:], func=ACTF.Ln, bias=1.0,
                            accum_out=lsums[:, b:b + 1])

            # ---- endgame ----
            ALUm = ALU
            v = finp.tile([128, B_LOC], F32)
            nc.vector.tensor_tensor(out=v[:], in0=xsums[:], in1=absums[:],
                                    op=ALUm.add)
            v2 = finp.tile([128, B_LOC], F32)
            nc.vector.scalar_tensor_tensor(
                out=v2[:], in0=v[:], scalar=0.5, in1=lsums[:],
                op0=ALUm.mult, op1=ALUm.add)
            spv = finp.tile([128, 1], F32)
            nc.vector.tensor_reduce(
                out=spv[:], in_=v2[:], axis=mybir.AxisListType.X, op=ALUm.add)

            mred = finp.tile([S, 1], F32)
            nc.vector.tensor_reduce(
                out=mred[:], in_=mask_cols[:], axis=mybir.AxisListType.X, op=ALUm.add)

            rgs = finp.tile([1, 1], F32)
            if "squares" not in ablate:
                rgsb = finp.tile([1, F], F32)
                for k in range(3):
                    nc.scalar.copy(rgsb[:, k * 512:(k + 1) * 512], rg_ps[k][:])
                nc.vector.tensor_reduce(
                    out=rgs[:], in_=rgsb[:], axis=mybir.AxisListType.X, op=ALUm.add)
            else:
                nc.gpsimd.memset(rgs[:], 0.0)

            fin_ps = psf.tile([1, 4], F32)
            nc.tensor.matmul(fin_ps[:, 0:1], spv[:], ones[:], start=True, stop=True)
            nc.tensor.matmul(fin_ps[:, 2:3], mred[:], ones[:S, :], start=True, stop=True)

            out_sb = finp.tile([1, 4], F32)
            nc.scalar.copy(out_sb[:, 0:1], fin_ps[:, 0:1])
            nc.scalar.copy(out_sb[:, 1:2], rgs[:])
            nc.scalar.copy(out_sb[:, 2:3], fin_ps[:, 2:3])
            nc.gpsimd.memset(out_sb[:, 3:4], 0.0)
            nc.sync.dma_start(outp.ap(), out_sb[:])

    nc.compile()

    # Collapse the act-table loads: every activation used (square, abs, exp,
    # ln, copy, identity) lives in set 6 = natural_log_exp_and_others, but the
    # greedy inserter alternates sets 0/5 (one reload per iteration, ~1.3us
    # each). Pin the first load to set 6 and drop the rest (they carry no
    # sync info).
    first = True
    for bb in nc.m.functions[0].blocks:
        keep = []
        for inst in bb.instructions:
            if isinstance(inst, mybir.InstLoadActFuncSet):
                si = inst.sync_info
                assert not (si and (si.on_wait or si.on_update))
                if first:
                    inst.act_func_set_id = 6
                    first = False
                    keep.append(inst)
            else:
                keep.append(inst)
        if len(keep) != len(bb.instructions):
            il = bb.instructions
            il[:] = keep
    return nc


def _get_nc():
    if "nc" not in _CACHE:
        _CACHE["nc"] = _build_nc()
    return _CACHE["nc"]


def _get_runner():
    """Persistent jitted 8-core runner for the production build."""
    if "runner" in _CACHE:
        return _CACHE["runner"]
    _CACHE["runner"] = _make_runner(_get_nc())
    return _CACHE["runner"]


def _make_runner(nc):
    """Jitted 8-core runner (mirrors bass2jax.run_bass_via_pjrt)."""
    import jax
    from jax.sharding import Mesh, PartitionSpec
    from jax.experimental.shard_map import shard_map
    import concourse.mybir as mybir
    from concourse import bass2jax

    bass2jax.install_neuronx_cc_hook()

    partition_name = (nc.partition_id_tensor.name
                      if nc.partition_id_tensor else None)
    in_names, out_names, out_avals, zero_outs = [], [], [], []
    for alloc in nc.m.functions[0].allocations:
        if not isinstance(alloc, mybir.MemoryLocationSet):
            continue
        name = alloc.memorylocations[0].name
        if alloc.kind == "ExternalInput":
            if name != partition_name:
                in_names.append(name)
        elif alloc.kind == "ExternalOutput":
            out_names.append(name)
            shape = tuple(alloc.tensor_shape)
            dtype = mybir.dt.np(alloc.dtype)
            out_avals.append(jax.core.ShapedArray(shape, dtype))
            zero_outs.append(np.zeros(shape, dtype))
    n_params = len(in_names)
    all_names = in_names + out_names
    if partition_name is not None:
        all_names = all_names + [partition_name]

    def _body(*args):
        operands = list(args)
        if partition_name is not None:
            operands.append(bass2jax.partition_id_tensor())
        outs = bass2jax._bass_exec_p.bind(
            *operands,
            out_avals=tuple(out_avals),
            in_names=tuple(all_names),
            out_names=tuple(out_names),
            lowering_input_output_aliases=(),
            sim_require_finite=True,
            sim_require_nnan=True,
            nc=nc,
        )
        return tuple(outs)

    devices = jax.devices()[:N_CORES]
    mesh = Mesh(np.asarray(devices), ("core",))
    n_outs = len(out_names)
    sharded = jax.jit(
        shard_map(_body, mesh=mesh,
                  in_specs=(PartitionSpec("core"),) * (n_params + n_outs),
                  out_specs=(PartitionSpec("core"),) * n_outs,
                  check_rep=False),
        donate_argnums=tuple(range(n_params, n_params + n_outs)),
        keep_unused=True,
    )
    return {
        "fn": sharded, "mesh": mesh, "in_names": in_names,
        "out_names": out_names, "zero_outs": zero_outs, "n_params": n_params,
    }


def _shard_inputs(tri_feat_org, alpha, mask):
    """Concatenated per-core global inputs keyed by dram tensor name."""
    return {
        "feat": np.ascontiguousarray(tri_feat_org, dtype=np.float32),
        "alpha": np.ascontiguousarray(alpha, dtype=np.float32),
        "mask": np.ascontiguousarray(mask, dtype=np.float32),
    }


def _combine(partials_global):
    """partials_global: (8, 4) array of per-core partial scalars."""
    pg = np.asarray(partials_global, dtype=np.float64).reshape(N_CORES, 4)
    sp, rg, nt = pg[:, 0].sum(), pg[:, 1].sum(), pg[:, 2].sum()
    denom = float(B) * S * R * D
    return np.float32(sp / nt + 0.01 * rg / denom)


def kernel(tri_feat_org, alpha, mask):
    r = _get_runner()
    named = _shard_inputs(tri_feat_org, alpha, mask)
    args = [named[n] for n in r["in_names"]]
    zeros = [np.zeros((N_CORES * z.shape[0], *z.shape[1:]), z.dtype)
             for z in r["zero_outs"]]
    outs = r["fn"](*args, *zeros)
    part = np.asarray(outs[r["out_names"].index("partials")])
    return np.asarray(_combine(part), dtype=np.float32)



# revision 28
# speedup vs baseline: 575.1229x; 1.2067x over previous
"""Trainium2 Bass kernel for nn_KnowledgeCriterion (ComplEx-style loss).

Full (unsharded) inputs:
  tri_feat_org: (256, 128, 1536) f32
  alpha:        (256, 64, 128)   f32
  mask:         (256, 64)        f32
Output: scalar f32 loss.

Strategy: data-parallel over batch on 8 NeuronCores (32 batches/core).
Each core computes three partial scalars (softplus-sum, regul-dot, mask-sum);
host combines:  loss = sp/numtrue + 0.01 * regul_dot/(B*S*R*D).

The body is instruction-issue bound, not data bound, so batches are
processed in PAIRS: one DMA/square/negate per 2 batches, and the alpha /
softplus chain runs on stacked [2S=128, .] tiles (full partition use).
Features are staged to device DRAM as bf16 (host converts): halves HBM
traffic; the 2e-2 output tolerance dwarfs the resulting ~2e-5 error.

Per-pair on-chip pipeline (X = (R=128 part, 2 batches x F=1536 free)):
  s0[r] = sum_d [ r_re*(h_re*t_re + h_im*t_im) + r_im*(h_re*t_im - h_im*t_re) ]
     per batch: AD = h_re*[t_re|t_im] (Pool), EB = h_im*[t_re|t_im] (DVE),
     negate-in-place -r_im, then two fused STT accums -> p1, p2 -> negs0
  regul_dot += sum_r a2s[r] * sum_f X[r,f]^2
     ACT Square -> X2 (one op per pair); PE matmul (stationary=a2s col)
     accumulating in PSUM per batch
  score = -(a^3)*s0, a=(alpha-0.1)*mask; alpha pre-loaded for all batches
     in (2S, bp, R) layout, transposed via PE per pair
  softplus(score) = (score+|score|)/2 + ln(1+exp(-|score|))
     DVE scoreT (per batch, accum->xsums); ACT Abs/Exp/Ln per pair
"""
import numpy as np

B, S, R, F = 256, 64, 128, 1536
D = F // 6
N_CORES = 8
B_LOC = B // N_CORES
NP = B_LOC // 2  # batch pairs per core

_CACHE = {}


def _build_nc(loop_k=1, ablate=()):
    """Build the per-core program. loop_k > 1 wraps the whole body in a
    hardware For_i loop (timing-only variant: outputs stay correct because
    every rep restarts its accumulations)."""
    import contextlib
    import concourse.bacc as bacc
    import concourse.tile as tile
    import concourse.masks as masks
    from concourse import mybir

    F32 = mybir.dt.float32
    BF16 = mybir.dt.bfloat16
    ALU = mybir.AluOpType
    ACTF = mybir.ActivationFunctionType

    nc = bacc.Bacc("TRN2", target_bir_lowering=False, debug=False)
    feat = nc.dram_tensor("feat", [B_LOC, R, F], BF16, kind="ExternalInput")
    alph = nc.dram_tensor("alpha", [B_LOC, S, R], F32, kind="ExternalInput")
    msk = nc.dram_tensor("mask", [B_LOC, S], F32, kind="ExternalInput")
    outp = nc.dram_tensor("partials", [1, 4], F32, kind="ExternalOutput")

    with tile.TileContext(nc) as tc:
        with (
            tc.tile_pool(name="const", bufs=1) as constp,
            tc.tile_pool(name="xf", bufs=8) as xf,
            tc.tile_pool(name="x2", bufs=3) as x2p,
            tc.tile_pool(name="prod", bufs=8) as prod,
            tc.tile_pool(name="alp", bufs=8) as alp,
            tc.tile_pool(name="sco", bufs=12) as sco,
            tc.tile_pool(name="cols", bufs=12) as colsp,
            tc.tile_pool(name="accum", bufs=1) as accp,
            tc.tile_pool(name="fin", bufs=1) as finp,
            tc.tile_pool(name="pst", bufs=3, space="PSUM") as pst,
            tc.tile_pool(name="psr", bufs=1, space="PSUM") as psr,
            tc.tile_pool(name="psf", bufs=1, space="PSUM") as psf,
            tc.tile_pool(name="psm", bufs=1, space="PSUM") as psm,
        ):
            ident = constp.tile([128, 128], F32)
            masks.make_identity(nc, ident[:])
            ones = constp.tile([128, 1], F32)
            nc.gpsimd.memset(ones[:], 1.0)

            # accumulation buffers: xsums one column per batch,
            # absums/lsums one column per pair
            xsums = accp.tile([128, B_LOC], F32)
            lsums = accp.tile([128, NP], F32)
            absums = accp.tile([128, NP], F32)
            if "softplus" in ablate:
                for t in (lsums, xsums, absums):
                    nc.gpsimd.memset(t[:], 0.0)

            # one consolidated mask load (B_LOC,S) -> transpose -> (S,B_LOC)
            mask_nat = accp.tile([B_LOC, S], F32)
            nc.sync.dma_start(mask_nat[:], msk.ap())
            maskT_ps = psm.tile([S, B_LOC], F32, tag="maskT_ps")
            nc.tensor.transpose(maskT_ps[:], mask_nat[:], ident[:B_LOC, :B_LOC])
            mask_cols = accp.tile([S, B_LOC], F32)
            nc.vector.tensor_copy(mask_cols[:], maskT_ps[:])
            # stacked per-pair mask: column bp = [mask(2bp); mask(2bp+1)]
            mask2 = accp.tile([2 * S, NP], F32)
            nc.vector.tensor_copy(mask2[0:S, :], mask_cols[:, 0::2])
            nc.vector.tensor_copy(mask2[S:2 * S, :], mask_cols[:, 1::2])

            # persistent PSUM accumulators for regul (3 chunks of 512)
            rg_ps = [psr.tile([1, 512], F32, name=f"rg_ps{k}", tag=f"rg{k}")
                     for k in range(3)]

            # all alpha, loaded once: (2S, bp, R), batch parity on the
            # partition halves. Keeps the per-pair loop X-only so the big
            # X copies round-robin all DMA queues.
            alt2 = accp.tile([2 * S, NP, R], F32)
            for b in range(B_LOC):
                nc.sync.dma_start(
                    alt2[(b % 2) * S:(b % 2 + 1) * S, b // 2, :], alph.ap()[b])

            if loop_k > 1:
                loop_cm = tc.For_i(
                    0, loop_k, 1,
                    hint_engines=(mybir.EngineType.DVE, mybir.EngineType.Activation,
                                  mybir.EngineType.Pool, mybir.EngineType.PE,
                                  mybir.EngineType.SP))
            else:
                loop_cm = contextlib.nullcontext()
            with loop_cm:
                for bp in range(NP):
                    b0 = 2 * bp
                    # ---- load: one DMA for the batch pair, (R, 2, F) ----
                    X = xf.tile([R, 2, F], BF16)
                    nc.sync.dma_start(
                        X[:], feat.ap()[b0:b0 + 2].transpose([1, 0, 2]))

                    # ---- alpha side, both batches stacked on partitions ----
                    am2 = alp.tile([2 * S, R], F32, tag="am2")
                    nc.vector.tensor_scalar(
                        out=am2[:], in0=alt2[:, bp, :], scalar1=0.1,
                        scalar2=mask2[:, bp:bp + 1],
                        op0=ALU.subtract, op1=ALU.mult)
                    amT_ps = pst.tile([R, 2 * S], F32, tag="amT_ps")
                    nc.tensor.transpose(amT_ps[:], am2[:], ident[:])
                    amT = alp.tile([R, 2 * S], F32, tag="amT")
                    nc.scalar.copy(amT[:], amT_ps[:])

                    # a^2 with per-batch column-sum accumulators
                    a2T = sco.tile([R, 2 * S], F32, tag="a2T")
                    a2s = [colsp.tile([R, 1], F32, name=f"a2s{p}", tag=f"a2s{p}")
                           for p in (0, 1)]
                    for p in (0, 1):
                        nc.vector.scalar_tensor_tensor(
                            out=a2T[:, p * S:(p + 1) * S],
                            in0=amT[:, p * S:(p + 1) * S], scalar=1.0,
                            in1=amT[:, p * S:(p + 1) * S],
                            op0=ALU.mult, op1=ALU.mult, accum_out=a2s[p][:])
                    a3T = sco.tile([R, 2 * S], F32, tag="a3T")
                    nc.vector.tensor_tensor(out=a3T[:], in0=a2T[:], in1=amT[:],
                                            op=ALU.mult)

                    # ---- feature side: squares for regul (one op per pair;
                    # bf16 for full-rate PE; regul is a 1e-4-scale term) ----
                    X2 = x2p.tile([R, 2, F], BF16)
                    nc.scalar.activation(out=X2[:], in_=X[:], func=ACTF.Square)
                    for p in (0, 1):
                        a2sb = colsp.tile([R, 1], BF16, tag=f"a2sb{p}")
                        nc.scalar.copy(a2sb[:], a2s[p][:])
                        for k in range(3):
                            nc.tensor.matmul(
                                rg_ps[k][:], a2sb[:],
                                X2[:, p, k * 512:(k + 1) * 512],
                                start=(bp == 0 and p == 0),
                                stop=(bp == NP - 1 and p == 1))

                    # ---- feature side: s0 per batch; negate once per pair ----
                    # AD = h_re*[t_re|t_im]; EB = h_im*[t_re|t_im]
                    # p1 = sum(AD*[r_re|r_im]); X[h_im slot] <- -r_im so
                    # [negrim|r_re] is contiguous; p2 = sum(EB*[-r_im|r_re])
                    # negs0 = -(p1 + p2)
                    AD, EB = [], []
                    for p in (0, 1):
                        ADp = prod.tile([R, 2 * D], BF16, tag=f"AD{p}")
                        nc.gpsimd.tensor_tensor(
                            out=ADp[:],
                            in0=X[:, p, None, 0:D].broadcast_to([R, 2, D]),
                            in1=X[:, p, 4 * D:6 * D], op=ALU.mult)
                        AD.append(ADp)
                        EBp = prod.tile([R, 2 * D], BF16, tag=f"EB{p}")
                        nc.vector.tensor_tensor(
                            out=EBp[:],
                            in0=X[:, p, None, D:2 * D].broadcast_to([R, 2, D]),
                            in1=X[:, p, 4 * D:6 * D], op=ALU.mult)
                        EB.append(EBp)
                    nc.vector.tensor_scalar(
                        out=X[:, :, 1 * D:2 * D], in0=X[:, :, 3 * D:4 * D],
                        scalar1=-1.0, scalar2=0.0, op0=ALU.mult, op1=ALU.add)
                    negs0 = []
                    for p in (0, 1):
                        jA = prod.tile([R, 2 * D], BF16, tag=f"jA{p}")
                        p1 = colsp.tile([R, 1], F32, tag=f"p1{p}")
                        nc.vector.scalar_tensor_tensor(
                            out=jA[:], in0=AD[p][:], scalar=1.0,
                            in1=X[:, p, 2 * D:4 * D],
                            op0=ALU.mult, op1=ALU.mult, accum_out=p1[:])
                        jB = prod.tile([R, 2 * D], BF16, tag=f"jB{p}")
                        p2 = colsp.tile([R, 1], F32, tag=f"p2{p}")
                        nc.vector.scalar_tensor_tensor(
                            out=jB[:], in0=EB[p][:], scalar=1.0,
                            in1=X[:, p, 1 * D:3 * D],
                            op0=ALU.mult, op1=ALU.mult, accum_out=p2[:])
                        ns = colsp.tile([R, 1], F32, tag=f"negs0{p}")
                        nc.vector.scalar_tensor_tensor(
                            out=ns[:], in0=p1[:], scalar=-1.0, in1=p2[:],
                            op0=ALU.mult, op1=ALU.subtract)
                        negs0.append(ns)

                    # ---- score & softplus: score per batch, act ops per pair ----
                    scoreT = sco.tile([R, 2 * S], F32, tag="scoreT")
                    for p in (0, 1):
                        nc.vector.tensor_scalar(
                            out=scoreT[:, p * S:(p + 1) * S],
                            in0=a3T[:, p * S:(p + 1) * S],
                            scalar1=negs0[p][:], scalar2=0.0,
                            op0=ALU.mult, op1=ALU.add,
                            accum_out=xsums[:, b0 + p:b0 + p + 1])
                    mT = sco.tile([R, 2 * S], F32, tag="mT")
                    nc.scalar.activation(
                        out=mT[:], in_=scoreT[:], func=ACTF.Abs,
                        accum_out=absums[:, bp:bp + 1])
                    expT = sco.tile([R, 2 * S], F32, tag="expT")
                    nc.scalar.activation(out=expT[:], in_=mT[:], func=ACTF.Exp,
                                         scale=-1.0)
                    lnT = sco.tile([R, 2 * S], F32, tag="lnT")
                    nc.scalar.activation(
                        out=lnT[:], in_=expT[:], func=ACTF.Ln, bias=1.0,
                        accum_out=lsums[:, bp:bp + 1])

            # ---- endgame: sp = 0.5*(sum xsums + sum absums) + sum lsums ----
            ALUm = ALU
            rx = finp.tile([128, 1], F32)
            nc.vector.tensor_reduce(
                out=rx[:], in_=xsums[:], axis=mybir.AxisListType.X, op=ALUm.add)
            rab = finp.tile([128, 1], F32)
            nc.vector.tensor_reduce(
                out=rab[:], in_=absums[:], axis=mybir.AxisListType.X, op=ALUm.add)
            rl = finp.tile([128, 1], F32)
            nc.vector.tensor_reduce(
                out=rl[:], in_=lsums[:], axis=mybir.AxisListType.X, op=ALUm.add)
            vt = finp.tile([128, 1], F32)
            nc.vector.tensor_tensor(out=vt[:], in0=rx[:], in1=rab[:], op=ALUm.add)
            spv = finp.tile([128, 1], F32)
            nc.vector.scalar_tensor_tensor(
                out=spv[:], in0=vt[:], scalar=0.5, in1=rl[:],
                op0=ALUm.mult, op1=ALUm.add)

            mred = finp.tile([S, 1], F32)
            nc.vector.tensor_reduce(
                out=mred[:], in_=mask_cols[:], axis=mybir.AxisListType.X, op=ALUm.add)

            rgs = finp.tile([1, 1], F32)
            rgsb = finp.tile([1, F], F32)
            for k in range(3):
                nc.scalar.copy(rgsb[:, k * 512:(k + 1) * 512], rg_ps[k][:])
            nc.vector.tensor_reduce(
                out=rgs[:], in_=rgsb[:], axis=mybir.AxisListType.X, op=ALUm.add)

            fin_ps = psf.tile([1, 4], F32)
            nc.tensor.matmul(fin_ps[:, 0:1], spv[:], ones[:], start=True, stop=True)
            nc.tensor.matmul(fin_ps[:, 2:3], mred[:], ones[:S, :], start=True, stop=True)

            out_sb = finp.tile([1, 4], F32)
            nc.scalar.copy(out_sb[:, 0:1], fin_ps[:, 0:1])
            nc.scalar.copy(out_sb[:, 1:2], rgs[:])
            nc.scalar.copy(out_sb[:, 2:3], fin_ps[:, 2:3])
            nc.gpsimd.memset(out_sb[:, 3:4], 0.0)
            nc.sync.dma_start(outp.ap(), out_sb[:])

    nc.compile()

    # Collapse the act-table loads: every activation used (square, abs, exp,
    # ln, copy, identity) lives in set 6 = natural_log_exp_and_others, but the
    # greedy inserter alternates sets (one reload per iteration, ~1.3us
    # each). Pin the first load to set 6 and drop the rest (they carry no
    # sync info).
    first = True
    for bb in nc.m.functions[0].blocks:
        keep = []
        for inst in bb.instructions:
            if isinstance(inst, mybir.InstLoadActFuncSet):
                si = inst.sync_info
                assert not (si and (si.on_wait or si.on_update))
                if first:
                    inst.act_func_set_id = 6
                    first = False
                    keep.append(inst)
            else:
                keep.append(inst)
        if len(keep) != len(bb.instructions):
            il = bb.instructions
            il[:] = keep
    return nc


def _get_nc():
    if "nc" not in _CACHE:
        _CACHE["nc"] = _build_nc()
    return _CACHE["nc"]


def _get_runner():
    """Persistent jitted 8-core runner for the production build."""
    if "runner" in _CACHE:
        return _CACHE["runner"]
    _CACHE["runner"] = _make_runner(_get_nc())
    return _CACHE["runner"]


def _make_runner(nc):
    """Jitted 8-core runner (mirrors bass2jax.run_bass_via_pjrt)."""
    import jax
    from jax.sharding import Mesh, PartitionSpec
    from jax.experimental.shard_map import shard_map
    import concourse.mybir as mybir
    from concourse import bass2jax

    bass2jax.install_neuronx_cc_hook()

    partition_name = (nc.partition_id_tensor.name
                      if nc.partition_id_tensor else None)
    in_names, out_names, out_avals, zero_outs = [], [], [], []
    for alloc in nc.m.functions[0].allocations:
        if not isinstance(alloc, mybir.MemoryLocationSet):
            continue
        name = alloc.memorylocations[0].name
        if alloc.kind == "ExternalInput":
            if name != partition_name:
                in_names.append(name)
        elif alloc.kind == "ExternalOutput":
            out_names.append(name)
            shape = tuple(alloc.tensor_shape)
            dtype = mybir.dt.np(alloc.dtype)
            out_avals.append(jax.core.ShapedArray(shape, dtype))
            zero_outs.append(np.zeros(shape, dtype))
    n_params = len(in_names)
    all_names = in_names + out_names
    if partition_name is not None:
        all_names = all_names + [partition_name]

    def _body(*args):
        operands = list(args)
        if partition_name is not None:
            operands.append(bass2jax.partition_id_tensor())
        outs = bass2jax._bass_exec_p.bind(
            *operands,
            out_avals=tuple(out_avals),
            in_names=tuple(all_names),
            out_names=tuple(out_names),
            lowering_input_output_aliases=(),
            sim_require_finite=True,
            sim_require_nnan=True,
            nc=nc,
        )
        return tuple(outs)

    devices = jax.devices()[:N_CORES]
    mesh = Mesh(np.asarray(devices), ("core",))
    n_outs = len(out_names)
    sharded = jax.jit(
        shard_map(_body, mesh=mesh,
                  in_specs=(PartitionSpec("core"),) * (n_params + n_outs),
                  out_specs=(PartitionSpec("core"),) * n_outs,
                  check_rep=False),
        donate_argnums=tuple(range(n_params, n_params + n_outs)),
        keep_unused=True,
    )
    return {
        "fn": sharded, "mesh": mesh, "in_names": in_names,
        "out_names": out_names, "zero_outs": zero_outs, "n_params": n_params,
    }


def _shard_inputs(tri_feat_org, alpha, mask):
    """Concatenated per-core global inputs keyed by dram tensor name.
    Features are staged to device DRAM as bf16 (see module docstring)."""
    import ml_dtypes
    return {
        "feat": np.ascontiguousarray(tri_feat_org).astype(ml_dtypes.bfloat16),
        "alpha": np.ascontiguousarray(alpha, dtype=np.float32),
        "mask": np.ascontiguousarray(mask, dtype=np.float32),
    }


def _combine(partials_global):
    """partials_global: (8, 4) array of per-core partial scalars."""
    pg = np.asarray(partials_global, dtype=np.float64).reshape(N_CORES, 4)
    sp, rg, nt = pg[:, 0].sum(), pg[:, 1].sum(), pg[:, 2].sum()
    denom = float(B) * S * R * D
    return np.float32(sp / nt + 0.01 * rg / denom)


def kernel(tri_feat_org, alpha, mask):
    r = _get_runner()
    named = _shard_inputs(tri_feat_org, alpha, mask)
    args = [named[n] for n in r["in_names"]]
    zeros = [np.zeros((N_CORES * z.shape[0], *z.shape[1:]), z.dtype)
             for z in r["zero_outs"]]
    outs = r["fn"](*args, *zeros)
    part = np.asarray(outs[r["out_names"].index("partials")])
    return np.asarray(_combine(part), dtype=np.float32)


# revision 41
# speedup vs baseline: 664.1460x; 1.1548x over previous
"""Trainium2 Bass kernel for nn_KnowledgeCriterion (ComplEx-style loss).

Full (unsharded) inputs:
  tri_feat_org: (256, 128, 1536) f32
  alpha:        (256, 64, 128)   f32
  mask:         (256, 64)        f32
Output: scalar f32 loss.

Strategy: data-parallel over batch on 8 NeuronCores (32 batches/core).
Each core computes three partial scalars (softplus-sum, regul-dot, mask-sum);
host combines:  loss = sp/numtrue + 0.01 * regul_dot/(B*S*R*D).

Batches are processed in PAIRS (alpha/softplus chains on stacked
[2S=128, .] tiles for full partition use; one square/negate per pair)
with feature DMA in QUAD-batch copies and alpha pre-loaded in two
stepped-slice copies, minimizing both bytes and copy count.
Features are staged to device DRAM as bf16 (host converts): halves HBM
traffic; the 2e-2 output tolerance dwarfs the resulting ~2e-5 error.

Per-pair on-chip pipeline (X = (R=128 part, 2 batches x F=1536 free)):
  s0[r] = sum_d [ r_re*(h_re*t_re + h_im*t_im) + r_im*(h_re*t_im - h_im*t_re) ]
     per batch: AD = h_re*[t_re|t_im] (Pool), EB = h_im*[t_re|t_im] (DVE),
     negate-in-place -r_im, then two fused STT accums -> p1, p2 -> negs0
  regul_dot += sum_r a2s[r] * sum_f X[r,f]^2
     ACT Square -> X2 (one op per pair); PE matmul (stationary=a2s col)
     accumulating in PSUM per batch
  score = -(a^3)*s0, a=(alpha-0.1)*mask; alpha pre-loaded for all batches
     in (2S, bp, R) layout, transposed via PE per pair
  softplus(score) = (score+|score|)/2 + ln(1+exp(-|score|))
     DVE scoreT (per batch, accum->xsums); ACT Abs/Exp/Ln per pair
"""
import numpy as np

B, S, R, F = 256, 64, 128, 1536
D = F // 6
N_CORES = 8
B_LOC = B // N_CORES
NP = B_LOC // 2  # batch pairs per core

_CACHE = {}


def _build_nc(loop_k=1, ablate=()):
    """Build the per-core program. loop_k > 1 wraps the whole body in a
    hardware For_i loop (timing-only variant: outputs stay correct because
    every rep restarts its accumulations)."""
    import contextlib
    import concourse.bacc as bacc
    import concourse.tile as tile
    import concourse.masks as masks
    from concourse import mybir

    F32 = mybir.dt.float32
    BF16 = mybir.dt.bfloat16
    ALU = mybir.AluOpType
    ACTF = mybir.ActivationFunctionType

    nc = bacc.Bacc("TRN2", target_bir_lowering=False, debug=False)
    feat = nc.dram_tensor("feat", [B_LOC, R, F], BF16, kind="ExternalInput")
    alph = nc.dram_tensor("alpha", [B_LOC, S, R], F32, kind="ExternalInput")
    msk = nc.dram_tensor("mask", [B_LOC, S], F32, kind="ExternalInput")
    outp = nc.dram_tensor("partials", [1, 4], F32, kind="ExternalOutput")

    with tile.TileContext(nc) as tc:
        with (
            tc.tile_pool(name="const", bufs=1) as constp,
            tc.tile_pool(name="xf", bufs=4) as xf,
            tc.tile_pool(name="x2", bufs=3) as x2p,
            tc.tile_pool(name="prod", bufs=8) as prod,
            tc.tile_pool(name="alp", bufs=8) as alp,
            tc.tile_pool(name="sco", bufs=12) as sco,
            tc.tile_pool(name="cols", bufs=12) as colsp,
            tc.tile_pool(name="accum", bufs=1) as accp,
            tc.tile_pool(name="fin", bufs=1) as finp,
            tc.tile_pool(name="pst", bufs=3, space="PSUM") as pst,
            tc.tile_pool(name="psr", bufs=1, space="PSUM") as psr,
            tc.tile_pool(name="psf", bufs=1, space="PSUM") as psf,
            tc.tile_pool(name="psm", bufs=1, space="PSUM") as psm,
        ):
            ident = constp.tile([128, 128], F32)
            masks.make_identity(nc, ident[:])
            ones = constp.tile([128, 1], F32)
            nc.gpsimd.memset(ones[:], 1.0)

            # accumulation buffers: xsums one column per batch,
            # absums/lsums one column per pair
            xsums = accp.tile([128, B_LOC], F32)
            lsums = accp.tile([128, NP], F32)
            absums = accp.tile([128, NP], F32)
            if "softplus" in ablate:
                for t in (lsums, xsums, absums):
                    nc.gpsimd.memset(t[:], 0.0)

            # one consolidated mask load (B_LOC,S) -> transpose -> (S,B_LOC)
            mask_nat = accp.tile([B_LOC, S], F32)
            nc.sync.dma_start(mask_nat[:], msk.ap())
            maskT_ps = psm.tile([S, B_LOC], F32, tag="maskT_ps")
            nc.tensor.transpose(maskT_ps[:], mask_nat[:], ident[:B_LOC, :B_LOC])
            mask_cols = accp.tile([S, B_LOC], F32)
            nc.vector.tensor_copy(mask_cols[:], maskT_ps[:])
            # stacked per-pair mask: column bp = [mask(2bp); mask(2bp+1)]
            mask2 = accp.tile([2 * S, NP], F32)
            nc.vector.tensor_copy(mask2[0:S, :], mask_cols[:, 0::2])
            nc.vector.tensor_copy(mask2[S:2 * S, :], mask_cols[:, 1::2])

            # persistent PSUM accumulators for regul (3 chunks of 512)
            rg_ps = [psr.tile([1, 512], F32, name=f"rg_ps{k}", tag=f"rg{k}")
                     for k in range(3)]

            # all alpha, loaded once: (2S, bp, R), batch parity on the
            # partition halves. Keeps the per-pair loop X-only so the big
            # X copies round-robin all DMA queues.
            alt2 = accp.tile([2 * S, NP, R], F32)
            nc.sync.dma_start(alt2[0:S], alph.ap()[0::2].transpose([1, 0, 2]))
            nc.sync.dma_start(alt2[S:2 * S], alph.ap()[1::2].transpose([1, 0, 2]))

            # The whole alpha chain is loop-invariant: precompute a^3 (as
            # (R, 2S) blocks per pair) and the bf16 a^2 column sums for all
            # batches once, so the streaming loop has no alpha-side
            # DVE->PE->ACT->DVE dependency chain at all.
            a3T_all = accp.tile([R, NP * 2 * S], F32)
            a2sb_all = accp.tile([R, B_LOC], BF16)
            for bp in range(NP):
                am2 = alp.tile([2 * S, R], F32, tag="am2")
                nc.vector.tensor_scalar(
                    out=am2[:], in0=alt2[:, bp, :], scalar1=0.1,
                    scalar2=mask2[:, bp:bp + 1],
                    op0=ALU.subtract, op1=ALU.mult)
                amT_ps = pst.tile([R, 2 * S], F32, tag="amT_ps")
                nc.tensor.transpose(amT_ps[:], am2[:], ident[:])
                amT = alp.tile([R, 2 * S], F32, tag="amT")
                nc.scalar.copy(amT[:], amT_ps[:])
                a2T = sco.tile([R, 2 * S], F32, tag="a2T")
                for p in (0, 1):
                    a2s = colsp.tile([R, 1], F32, name=f"a2s{p}", tag=f"a2s{p}")
                    nc.vector.scalar_tensor_tensor(
                        out=a2T[:, p * S:(p + 1) * S],
                        in0=amT[:, p * S:(p + 1) * S], scalar=1.0,
                        in1=amT[:, p * S:(p + 1) * S],
                        op0=ALU.mult, op1=ALU.mult, accum_out=a2s[:])
                    nc.vector.tensor_copy(
                        a2sb_all[:, 2 * bp + p:2 * bp + p + 1], a2s[:])
                nc.vector.tensor_tensor(
                    out=a3T_all[:, bp * 2 * S:(bp + 1) * 2 * S],
                    in0=a2T[:], in1=amT[:], op=ALU.mult)

            if loop_k > 1:
                loop_cm = tc.For_i(
                    0, loop_k, 1,
                    hint_engines=(mybir.EngineType.DVE, mybir.EngineType.Activation,
                                  mybir.EngineType.Pool, mybir.EngineType.PE,
                                  mybir.EngineType.SP))
            else:
                loop_cm = contextlib.nullcontext()
            with loop_cm:
                for bq in range(NP // 2):
                  # ---- load: one DMA for FOUR batches, (R, 4, F); DMA here
                  # is copy-count-bound (~1.7us/copy regardless of size), so
                  # fewer, bigger copies win. Negate-in-place also runs once
                  # per quad. The rest of the body runs per batch pair.
                  Xq = xf.tile([R, 4, F], BF16)
                  if bq == 0:
                      # fill-latency cut: first quad arrives as 4 parallel
                      # single-batch copies (~6us latency each) instead of one
                      # ~22us copy; steady-state quads stay single-copy.
                      for q in range(4):
                          nc.sync.dma_start(Xq[:, q, :], feat.ap()[q])
                  else:
                      nc.sync.dma_start(
                          Xq[:], feat.ap()[4 * bq:4 * bq + 4].transpose([1, 0, 2]))
                  nc.vector.tensor_scalar(
                      out=Xq[:, :, 1 * D:2 * D], in0=Xq[:, :, 3 * D:4 * D],
                      scalar1=-1.0, scalar2=0.0, op0=ALU.mult, op1=ALU.add)
                  for half in (0, 1):
                    bp = 2 * bq + half
                    b0 = 2 * bp
                    X = Xq[:, 2 * half:2 * half + 2, :]

                    a3T = a3T_all[:, bp * 2 * S:(bp + 1) * 2 * S]

                    # ---- feature side: squares for regul (one op per pair;
                    # bf16 for full-rate PE; regul is a 1e-4-scale term) ----
                    X2 = x2p.tile([R, 2, F], BF16)
                    nc.scalar.activation(out=X2[:], in_=X[:], func=ACTF.Square)
                    for p in (0, 1):
                        for k in range(3):
                            nc.tensor.matmul(
                                rg_ps[k][:],
                                a2sb_all[:, b0 + p:b0 + p + 1],
                                X2[:, p, k * 512:(k + 1) * 512],
                                start=(bp == 0 and p == 0),
                                stop=(bp == NP - 1 and p == 1))

                    # ---- feature side: s0 per batch; negate once per pair ----
                    # AD = h_re*[t_re|t_im]; EB = h_im*[t_re|t_im]
                    # p1 = sum(AD*[r_re|r_im]); X[h_im slot] <- -r_im so
                    # [negrim|r_re] is contiguous; p2 = sum(EB*[-r_im|r_re])
                    # negs0 = -(p1 + p2)
                    AD, EB = [], []
                    for p in (0, 1):
                        ADp = prod.tile([R, 2 * D], BF16, tag=f"AD{p}")
                        nc.gpsimd.tensor_tensor(
                            out=ADp[:],
                            in0=X[:, p, None, 0:D].broadcast_to([R, 2, D]),
                            in1=X[:, p, 4 * D:6 * D], op=ALU.mult)
                        AD.append(ADp)
                        EBp = prod.tile([R, 2 * D], BF16, tag=f"EB{p}")
                        nc.vector.tensor_tensor(
                            out=EBp[:],
                            in0=X[:, p, None, D:2 * D].broadcast_to([R, 2, D]),
                            in1=X[:, p, 4 * D:6 * D], op=ALU.mult)
                        EB.append(EBp)
                    nc.vector.tensor_scalar(
                        out=X[:, :, 1 * D:2 * D], in0=X[:, :, 3 * D:4 * D],
                        scalar1=-1.0, scalar2=0.0, op0=ALU.mult, op1=ALU.add)
                    negs0 = []
                    for p in (0, 1):
                        jA = prod.tile([R, 2 * D], BF16, tag=f"jA{p}")
                        p1 = colsp.tile([R, 1], F32, tag=f"p1{p}")
                        nc.vector.scalar_tensor_tensor(
                            out=jA[:], in0=AD[p][:], scalar=1.0,
                            in1=X[:, p, 2 * D:4 * D],
                            op0=ALU.mult, op1=ALU.mult, accum_out=p1[:])
                        jB = prod.tile([R, 2 * D], BF16, tag=f"jB{p}")
                        p2 = colsp.tile([R, 1], F32, tag=f"p2{p}")
                        nc.vector.scalar_tensor_tensor(
                            out=jB[:], in0=EB[p][:], scalar=1.0,
                            in1=X[:, p, 1 * D:3 * D],
                            op0=ALU.mult, op1=ALU.mult, accum_out=p2[:])
                        ns = colsp.tile([R, 1], F32, tag=f"negs0{p}")
                        nc.vector.scalar_tensor_tensor(
                            out=ns[:], in0=p1[:], scalar=-1.0, in1=p2[:],
                            op0=ALU.mult, op1=ALU.subtract)
                        negs0.append(ns)

                    # ---- score & softplus: score per batch, act ops per pair ----
                    scoreT = sco.tile([R, 2 * S], F32, tag="scoreT")
                    for p in (0, 1):
                        nc.vector.tensor_scalar(
                            out=scoreT[:, p * S:(p + 1) * S],
                            in0=a3T[:, p * S:(p + 1) * S],
                            scalar1=negs0[p][:], scalar2=0.0,
                            op0=ALU.mult, op1=ALU.add,
                            accum_out=xsums[:, b0 + p:b0 + p + 1])
                    mT = sco.tile([R, 2 * S], F32, tag="mT")
                    nc.scalar.activation(
                        out=mT[:], in_=scoreT[:], func=ACTF.Abs,
                        accum_out=absums[:, bp:bp + 1])
                    expT = sco.tile([R, 2 * S], F32, tag="expT")
                    nc.scalar.activation(out=expT[:], in_=mT[:], func=ACTF.Exp,
                                         scale=-1.0)
                    lnT = sco.tile([R, 2 * S], F32, tag="lnT")
                    nc.scalar.activation(
                        out=lnT[:], in_=expT[:], func=ACTF.Ln, bias=1.0,
                        accum_out=lsums[:, bp:bp + 1])

            # ---- endgame: sp = 0.5*(sum xsums + sum absums) + sum lsums ----
            ALUm = ALU
            rx = finp.tile([128, 1], F32)
            nc.vector.tensor_reduce(
                out=rx[:], in_=xsums[:], axis=mybir.AxisListType.X, op=ALUm.add)
            rab = finp.tile([128, 1], F32)
            nc.vector.tensor_reduce(
                out=rab[:], in_=absums[:], axis=mybir.AxisListType.X, op=ALUm.add)
            rl = finp.tile([128, 1], F32)
            nc.vector.tensor_reduce(
                out=rl[:], in_=lsums[:], axis=mybir.AxisListType.X, op=ALUm.add)
            vt = finp.tile([128, 1], F32)
            nc.vector.tensor_tensor(out=vt[:], in0=rx[:], in1=rab[:], op=ALUm.add)
            spv = finp.tile([128, 1], F32)
            nc.vector.scalar_tensor_tensor(
                out=spv[:], in0=vt[:], scalar=0.5, in1=rl[:],
                op0=ALUm.mult, op1=ALUm.add)

            mred = finp.tile([S, 1], F32)
            nc.vector.tensor_reduce(
                out=mred[:], in_=mask_cols[:], axis=mybir.AxisListType.X, op=ALUm.add)

            rgs = finp.tile([1, 1], F32)
            rgsb = finp.tile([1, F], F32)
            for k in range(3):
                nc.scalar.copy(rgsb[:, k * 512:(k + 1) * 512], rg_ps[k][:])
            nc.vector.tensor_reduce(
                out=rgs[:], in_=rgsb[:], axis=mybir.AxisListType.X, op=ALUm.add)

            fin_ps = psf.tile([1, 4], F32)
            nc.tensor.matmul(fin_ps[:, 0:1], spv[:], ones[:], start=True, stop=True)
            nc.tensor.matmul(fin_ps[:, 2:3], mred[:], ones[:S, :], start=True, stop=True)

            out_sb = finp.tile([1, 4], F32)
            nc.scalar.copy(out_sb[:, 0:1], fin_ps[:, 0:1])
            nc.scalar.copy(out_sb[:, 1:2], rgs[:])
            nc.scalar.copy(out_sb[:, 2:3], fin_ps[:, 2:3])
            nc.gpsimd.memset(out_sb[:, 3:4], 0.0)
            nc.sync.dma_start(outp.ap(), out_sb[:])

    nc.compile()

    # Collapse the act-table loads: every activation used (square, abs, exp,
    # ln, copy, identity) lives in set 6 = natural_log_exp_and_others, but the
    # greedy inserter alternates sets (one reload per iteration, ~1.3us
    # each). Pin the first load to set 6 and drop the rest (they carry no
    # sync info).
    first = True
    for bb in nc.m.functions[0].blocks:
        keep = []
        for inst in bb.instructions:
            if isinstance(inst, mybir.InstLoadActFuncSet):
                si = inst.sync_info
                assert not (si and (si.on_wait or si.on_update))
                if first:
                    inst.act_func_set_id = 6
                    first = False
                    keep.append(inst)
            else:
                keep.append(inst)
        if len(keep) != len(bb.instructions):
            il = bb.instructions
            il[:] = keep
    return nc


def _get_nc():
    if "nc" not in _CACHE:
        _CACHE["nc"] = _build_nc()
    return _CACHE["nc"]


def _get_runner():
    """Persistent jitted 8-core runner for the production build."""
    if "runner" in _CACHE:
        return _CACHE["runner"]
    _CACHE["runner"] = _make_runner(_get_nc())
    return _CACHE["runner"]


def _make_runner(nc):
    """Jitted 8-core runner (mirrors bass2jax.run_bass_via_pjrt)."""
    import jax
    from jax.sharding import Mesh, PartitionSpec
    from jax.experimental.shard_map import shard_map
    import concourse.mybir as mybir
    from concourse import bass2jax

    bass2jax.install_neuronx_cc_hook()

    partition_name = (nc.partition_id_tensor.name
                      if nc.partition_id_tensor else None)
    in_names, out_names, out_avals, zero_outs = [], [], [], []
    for alloc in nc.m.functions[0].allocations:
        if not isinstance(alloc, mybir.MemoryLocationSet):
            continue
        name = alloc.memorylocations[0].name
        if alloc.kind == "ExternalInput":
            if name != partition_name:
                in_names.append(name)
        elif alloc.kind == "ExternalOutput":
            out_names.append(name)
            shape = tuple(alloc.tensor_shape)
            dtype = mybir.dt.np(alloc.dtype)
            out_avals.append(jax.core.ShapedArray(shape, dtype))
            zero_outs.append(np.zeros(shape, dtype))
    n_params = len(in_names)
    all_names = in_names + out_names
    if partition_name is not None:
        all_names = all_names + [partition_name]

    def _body(*args):
        operands = list(args)
        if partition_name is not None:
            operands.append(bass2jax.partition_id_tensor())
        outs = bass2jax._bass_exec_p.bind(
            *operands,
            out_avals=tuple(out_avals),
            in_names=tuple(all_names),
            out_names=tuple(out_names),
            lowering_input_output_aliases=(),
            sim_require_finite=True,
            sim_require_nnan=True,
            nc=nc,
        )
        return tuple(outs)

    devices = jax.devices()[:N_CORES]
    mesh = Mesh(np.asarray(devices), ("core",))
    n_outs = len(out_names)
    sharded = jax.jit(
        shard_map(_body, mesh=mesh,
                  in_specs=(PartitionSpec("core"),) * (n_params + n_outs),
                  out_specs=(PartitionSpec("core"),) * n_outs,
                  check_rep=False),
        donate_argnums=tuple(range(n_params, n_params + n_outs)),
        keep_unused=True,
    )
    return {
        "fn": sharded, "mesh": mesh, "in_names": in_names,
        "out_names": out_names, "zero_outs": zero_outs, "n_params": n_params,
    }


def _shard_inputs(tri_feat_org, alpha, mask):
    """Concatenated per-core global inputs keyed by dram tensor name.
    Features are staged to device DRAM as bf16 (see module docstring)."""
    import ml_dtypes
    return {
        "feat": np.ascontiguousarray(tri_feat_org).astype(ml_dtypes.bfloat16),
        "alpha": np.ascontiguousarray(alpha, dtype=np.float32),
        "mask": np.ascontiguousarray(mask, dtype=np.float32),
    }


def _combine(partials_global):
    """partials_global: (8, 4) array of per-core partial scalars."""
    pg = np.asarray(partials_global, dtype=np.float64).reshape(N_CORES, 4)
    sp, rg, nt = pg[:, 0].sum(), pg[:, 1].sum(), pg[:, 2].sum()
    denom = float(B) * S * R * D
    return np.float32(sp / nt + 0.01 * rg / denom)


def kernel(tri_feat_org, alpha, mask):
    r = _get_runner()
    named = _shard_inputs(tri_feat_org, alpha, mask)
    args = [named[n] for n in r["in_names"]]
    zeros = [np.zeros((N_CORES * z.shape[0], *z.shape[1:]), z.dtype)
             for z in r["zero_outs"]]
    outs = r["fn"](*args, *zeros)
    part = np.asarray(outs[r["out_names"].index("partials")])
    return np.asarray(_combine(part), dtype=np.float32)
